# revision 1
# baseline (speedup 1.0000x reference)
"""DSC layer (moe_routing) on 8 TRN2 NeuronCores, data-parallel over tokens.

Math (per token n):
  r0[nb]   = sum_d x[n,d]*g[d]*rW[nb,d]            (bf16 matmul)
  r_raw    = rs[n]*r0 - rs[n]*mu[n]*sg[nb] + c[nb] (LN folded into scalars)
  alpha    = softplus(clip(r_raw, +-10))
  top-8 of alpha via HW max8 + match_replace -> masked alpha (Zscat)
  q[n]     = tanh(S)/(S+eps), S = sum of top-8
  h_full   = x @ U_norm.T ; G = Zscat*q*h_full
  dyn      = G @ (V_norm * gamma)   (accumulated into the same PSUM as static)
  static   = gelu(x@W1.T) @ W2.T
All matmuls bf16 (fp32 accum). Stats (mean/var) computed in f32 via bn_stats.
All transposed layouts are prepared host-side (free); only math runs on device.
"""
import sys, os
sys.path.insert(0, "/opt/trn_rl_repo")
from contextlib import ExitStack
import numpy as np
import concourse.bass as bass
import concourse.mybir as mybir
from concourse import bacc
from concourse.tile import TileContext
from concourse.bass_utils import run_bass_kernel_spmd

F32 = mybir.dt.float32
BF16 = mybir.dt.bfloat16
AF = mybir.ActivationFunctionType
OP = mybir.AluOpType
AX = mybir.AxisListType

D, NB, H = 1024, 512, 4096
NCORE = 8
T = 1024          # tokens per core
P = 128
TI = T // P       # 8 token tiles
DK = D // P       # 8 contraction tiles over D
HJ = H // P       # 32 tiles over ffn hidden
NBJ = NB // P     # 4 tiles over basis dim
TAU = 10.0
EPS = 1e-6
GELU = (AF.Identity if os.environ.get("KERNEL_NO_GELU") else AF.Gelu)


def _build():
    nc = bacc.Bacc("TRN2", target_bir_lowering=False, debug=False, num_devices=NCORE)
    xt_e = nc.declare_dram_parameter("xt", [D, T], F32, isOutput=False)
    w1t_e = nc.declare_dram_parameter("w1t", [D, H], F32, isOutput=False)
    w2t_e = nc.declare_dram_parameter("w2t", [H, D], F32, isOutput=False)
    rwt_e = nc.declare_dram_parameter("rwt", [D, NB], F32, isOutput=False)
    ut_e = nc.declare_dram_parameter("ut", [D, NB], F32, isOutput=False)
    v_e = nc.declare_dram_parameter("v", [NB, D], F32, isOutput=False)
    gcol_e = nc.declare_dram_parameter("gcol", [P, DK], F32, isOutput=False)
    bcol_e = nc.declare_dram_parameter("bcol", [P, DK], F32, isOutput=False)
    rb_e = nc.declare_dram_parameter("rb", [1, NB], F32, isOutput=False)
    gam_e = nc.declare_dram_parameter("gam", [1, D], F32, isOutput=False)
    eye_e = nc.declare_dram_parameter("eye", [P, P], F32, isOutput=False)
    out_e = nc.declare_dram_parameter("out", [T, D], F32, isOutput=True)

    xt_v = xt_e[:].rearrange("(ko p) t -> p ko t", p=P)      # [128, DK, T]
    w1t_v = w1t_e[:].rearrange("(ko p) h -> p ko h", p=P)    # [128, DK, H]
    w2t_v = w2t_e[:].rearrange("(ho p) d -> p ho d", p=P)    # [128, HJ, D]
    rwt_v = rwt_e[:].rearrange("(ko p) n -> p ko n", p=P)    # [128, DK, NB]
    ut_v = ut_e[:].rearrange("(ko p) n -> p ko n", p=P)      # [128, DK, NB]
    v_v = v_e[:].rearrange("(no p) d -> p no d", p=P)        # [128, NBJ, D]
    out_v = out_e[:].rearrange("(to p) d -> p to d", p=P)    # [128, TI, D]

    with TileContext(nc) as tc, ExitStack() as ctx:
        const = ctx.enter_context(tc.tile_pool(name="const", bufs=1))
        persist = ctx.enter_context(tc.tile_pool(name="persist", bufs=1))

        ones_row = const.tile([1, P], BF16)
        nc.vector.memset(ones_row[:], 1.0)
        ones_bc = const.tile([P, P], BF16)
        nc.vector.memset(ones_bc[:], 1.0)
        epsb = const.tile([P, 1], F32)
        nc.vector.memset(epsb[:], 1e-5)
        gcol = const.tile([P, DK], F32)
        bcol = const.tile([P, DK], F32)
        nc.sync.dma_start(gcol[:], gcol_e[:])
        nc.sync.dma_start(bcol[:], bcol_e[:])
        ident = const.tile([P, P], BF16)

        xtb = persist.tile([P, DK, T], BF16)      # 16K/part
        vs = persist.tile([P, NBJ, D], BF16)      # 8K
        gt = persist.tile([P, NBJ, T], BF16)      # 8K
        gall = persist.tile([P, TI, NB], BF16)    # 8K  (G, pre-transpose)
        zsall = persist.tile([P, TI, NB], BF16)   # 8K  (masked alpha)
        hfall = persist.tile([P, TI, NB], BF16)   # 8K  (h_full)
        rs_t = persist.tile([P, TI], F32)
        mrs_t = persist.tile([P, TI], F32)
        sall = persist.tile([P, TI], F32)
        thall = persist.tile([P, TI], F32)

        with tc.tile_pool(name="pares", bufs=1) as pares:
            wg = pares.tile([P, DK, NB], BF16)
            un = pares.tile([P, DK, NB], BF16)
            sg_b = pares.tile([P, NB], F32)
            c_b = pares.tile([P, NB], F32)

            pp0_ctx = ExitStack()
            with tc.tile_pool(name="p0", bufs=1) as p0, \
                 tc.tile_pool(name="p0s", bufs=2) as p0s, \
                 tc.tile_pool(name="p0b", bufs=2) as p0b, \
                 tc.tile_pool(name="pa", bufs=3) as pa, \
                 tc.tile_pool(name="pa_sm", bufs=2) as pa_sm, \
                 tc.tile_pool(name="ppa", bufs=6, space="PSUM") as ppa:
                # ---- bulk DMAs, ordered by need: x first, tables behind ----
                pp0 = pp0_ctx.enter_context(
                    tc.tile_pool(name="pp0", bufs=2, space="PSUM"))
                eyef = p0.tile([P, P], F32, tag="eyef")
                nc.sync.dma_start(eyef[:], eye_e[:])
                nc.gpsimd.tensor_copy(ident[:], eyef[:])
                rwtf = p0s.tile([P, DK, NB], F32, tag="stageB")
                nc.sync.dma_start(rwtf[:], rwt_v[:])
                rb_f = p0.tile([1, NB], F32, tag="rb_f")
                gam_f = p0.tile([1, D], F32, tag="gam_f")
                nc.sync.dma_start(rb_f[:], rb_e[:])
                nc.sync.dma_start(gam_f[:], gam_e[:])
                rb_row = p0.tile([1, NB], BF16, tag="rb_row")
                gam_row = p0.tile([1, D], BF16, tag="gam_row")
                nc.gpsimd.tensor_copy(rb_row[:], rb_f[:])
                nc.gpsimd.tensor_copy(gam_row[:], gam_f[:])

                utf = p0s.tile([P, DK, NB], F32, tag="stageB")
                nc.sync.dma_start(utf[:], ut_v[:])
                # DVE: router table casts first (rwt lands first), then x
                rwb = p0.tile([P, DK, NB], BF16, tag="rwb")
                for dk in range(DK):
                    nc.vector.tensor_copy(rwb[:, dk, :], rwtf[:, dk, :])
                for dk in range(DK):
                    xtf = p0b.tile([P, T], F32, tag="xtf")
                    nc.sync.dma_start(xtf[:], xt_v[:, dk, :])
                    nc.vector.tensor_copy(xtb[:, dk, :], xtf[:])
                for dk in range(DK):
                    nc.vector.tensor_scalar(wg[:, dk, :], rwtf[:, dk, :],
                                            gcol[:, dk : dk + 1], None, OP.mult)

                def emit_prep_mms():
                    gam_b = p0.tile([P, D], F32, tag="gam_b")
                    for half in range(2):
                        gam_ps = pp0.tile([P, 512], F32, tag="ps512")
                        nc.tensor.matmul(gam_ps[:], ones_row[:],
                                         gam_row[:, half * 512 : (half + 1) * 512],
                                         start=True, stop=True)
                        nc.vector.tensor_copy(
                            gam_b[:, half * 512 : (half + 1) * 512], gam_ps[:])
                    gbc = p0.tile([P, DK, P], BF16, tag="gbc")
                    bbc = p0.tile([P, DK, P], BF16, tag="bbc")
                    for dk in range(DK):
                        nc.gpsimd.tensor_copy(
                            gbc[:, dk, :], gcol[:, dk : dk + 1].to_broadcast([P, P]))
                        nc.gpsimd.tensor_copy(
                            bbc[:, dk, :], bcol[:, dk : dk + 1].to_broadcast([P, P]))
                    sg_ps = pp0.tile([P, NB], F32, tag="ps512")
                    for dk in range(DK):
                        nc.tensor.matmul(sg_ps[:], gbc[:, dk, :], rwb[:, dk, :],
                                         start=(dk == 0), stop=(dk == DK - 1))
                    nc.vector.tensor_copy(sg_b[:], sg_ps[:])
                    c_ps = pp0.tile([P, NB], F32, tag="ps512")
                    for dk in range(DK):
                        nc.tensor.matmul(c_ps[:], bbc[:, dk, :], rwb[:, dk, :],
                                         start=(dk == 0), stop=False)
                    nc.tensor.matmul(c_ps[:], ones_row[:], rb_row[:],
                                     start=False, stop=True)
                    nc.vector.tensor_copy(c_b[:], c_ps[:])
                    # U norms
                    nsq_ps = pp0.tile([P, NB], F32, tag="ps512")
                    for dk in range(DK):
                        usq = p0b.tile([P, NB], BF16, tag="usq")
                        useng = nc.vector if dk % 2 == 0 else nc.gpsimd
                        useng.tensor_tensor(usq[:], utf[:, dk, :],
                                            utf[:, dk, :], OP.mult)
                        nc.tensor.matmul(nsq_ps[:], ones_bc[:], usq[:],
                                         start=(dk == 0), stop=(dk == DK - 1))
                    rno = p0b.tile([P, NB], F32, tag="rno")
                    nc.scalar.activation(rno[:], nsq_ps[:], AF.Ln)
                    nc.scalar.activation(rno[:], rno[:], AF.Exp, scale=-0.5)
                    nc.vector.tensor_scalar_min(rno[:], rno[:], 1.0 / EPS)
                    for dk in range(DK):
                        ueng = nc.vector if dk % 2 == 0 else nc.gpsimd
                        ueng.tensor_tensor(un[:, dk, :], utf[:, dk, :],
                                           rno[:], OP.mult)
                    return gam_b

                gam_b = emit_prep_mms()
                # LN stats via ones-matmuls on x (PE) + transposes; no xn input
                sum_b = p0.tile([P, T], F32, tag="sum_b")
                sq_b = p0.tile([P, T], F32, tag="sq_b")
                for half in range(2):
                    hsl = slice(half * 512, (half + 1) * 512)
                    sps = pp0.tile([P, 512], F32, tag="ps512")
                    for dk in range(DK):
                        nc.tensor.matmul(sps[:], ones_bc[:], xtb[:, dk, hsl],
                                         start=(dk == 0), stop=(dk == DK - 1))
                    nc.vector.tensor_copy(sum_b[:, hsl], sps[:])
                for half in range(2):
                    hsl = slice(half * 512, (half + 1) * 512)
                    sps = pp0.tile([P, 512], F32, tag="ps512")
                    for dk in range(DK):
                        xsq = p0b.tile([P, 512], BF16, tag="xsq")
                        nc.vector.tensor_tensor(xsq[:], xtb[:, dk, hsl],
                                                xtb[:, dk, hsl], OP.mult)
                        nc.tensor.matmul(sps[:], ones_bc[:], xsq[:],
                                         start=(dk == 0), stop=(dk == DK - 1))
                    nc.vector.tensor_copy(sq_b[:, hsl], sps[:])
                mu_c = p0b.tile([P, TI], F32, tag="mu_c")
                sq_c = p0b.tile([P, TI], F32, tag="sq_c")
                for ti in range(TI):
                    tsl = slice(ti * P, (ti + 1) * P)
                    pts = pp0.tile([P, P], F32, tag="ps512")
                    nc.tensor.transpose(pts[:], sum_b[:, tsl], eyef[:])
                    nc.vector.tensor_copy(mu_c[:, ti : ti + 1], pts[:, 0:1])
                    ptq = pp0.tile([P, P], F32, tag="ps512")
                    nc.tensor.transpose(ptq[:], sq_b[:, tsl], eyef[:])
                    nc.vector.tensor_copy(sq_c[:, ti : ti + 1], ptq[:, 0:1])
                mu_all = p0b.tile([P, TI], F32, tag="mu_all")
                var_all = p0b.tile([P, TI], F32, tag="var_all")
                nc.vector.tensor_scalar_mul(mu_all[:], mu_c[:], 1.0 / D)
                nc.vector.tensor_scalar_mul(sq_c[:], sq_c[:], 1.0 / D)
                nc.vector.tensor_tensor(var_all[:], mu_all[:], mu_all[:], OP.mult)
                nc.vector.tensor_sub(var_all[:], sq_c[:], var_all[:])
                lnv = p0b.tile([P, TI], F32, tag="lnv")
                nc.scalar.activation(lnv[:], var_all[:], AF.Ln, bias=epsb[:])
                nc.scalar.activation(rs_t[:], lnv[:], AF.Exp, scale=-0.5)
                nc.vector.scalar_tensor_tensor(mrs_t[:], mu_all[:], -1.0,
                                               rs_t[:], OP.mult, OP.mult)

                # ---- A pass 1a: router matmuls + LN fixup ----
                rf_l, e_l, al_l = [], [], []

                for ti in range(TI):
                    tsl = slice(ti * P, (ti + 1) * P)
                    r0 = ppa.tile([P, NB], F32, tag="pA")
                    for dk in range(DK):
                        nc.tensor.matmul(r0[:], xtb[:, dk, tsl], wg[:, dk, :],
                                         start=(dk == 0), stop=(dk == DK - 1))
                    rf = pa.tile([P, NB], F32, tag="rf")
                    nc.vector.scalar_tensor_tensor(
                        rf[:], r0[:], rs_t[:, ti : ti + 1], c_b[:],
                        OP.mult, OP.add)
                    nc.vector.scalar_tensor_tensor(
                        rf[:], sg_b[:], mrs_t[:, ti : ti + 1], rf[:],
                        OP.mult, OP.add)
                    nc.gpsimd.tensor_scalar(rf[:], rf[:], TAU, -TAU,
                                            OP.min, OP.max)
                    rf_l.append(rf)

                # ---- A pass 1b: h_full matmuls (evict via ACT to SBUF) ----
                for ti in range(TI):
                    tsl = slice(ti * P, (ti + 1) * P)
                    hf = ppa.tile([P, NB], F32, tag="pA")
                    for dk in range(DK):
                        nc.tensor.matmul(hf[:], xtb[:, dk, tsl], un[:, dk, :],
                                         start=(dk == 0), stop=(dk == DK - 1))
                    nc.vector.tensor_copy(hfall[:, ti, :], hf[:])

                pp0_ctx.close()

                # ---- V norms (DVE; vf DMA behind tables on sync queue) ----
                vf = p0.tile([P, NBJ, D], F32, tag="stageA")
                nc.sync.dma_start(vf[:], v_v[:])
                vss = p0b.tile([P, NBJ], F32, tag="vss")
                rnv = p0b.tile([P, NBJ], F32, tag="rnv")
                for nbj in range(NBJ):
                    vsq = p0b.tile([P, D], F32, tag="vsq")
                    nc.gpsimd.tensor_tensor(vsq[:], vf[:, nbj, :], vf[:, nbj, :],
                                            OP.mult)
                    nc.vector.reduce_sum(vss[:, nbj : nbj + 1], vsq[:], axis=AX.X)
                nc.scalar.activation(rnv[:], vss[:], AF.Ln)
                nc.scalar.activation(rnv[:], rnv[:], AF.Exp, scale=-0.5)
                nc.vector.tensor_scalar_min(rnv[:], rnv[:], 1.0 / EPS)
                for nbj in range(NBJ):
                    nc.vector.scalar_tensor_tensor(
                        vs[:, nbj, :], vf[:, nbj, :], rnv[:, nbj : nbj + 1],
                        gam_b[:], OP.mult, OP.mult)

                # ---- A passes 2-6: softplus, top-8, q, G ----
                for ti in range(TI):
                    e_sb = pa.tile([P, NB], F32, tag="e_sb")
                    nc.scalar.activation(e_sb[:], rf_l[ti][:], AF.Exp)
                    e_l.append(e_sb)
                for ti in range(TI):
                    alpha = pa.tile([P, NB], F32, tag="alpha")
                    nc.scalar.activation(alpha[:], e_l[ti][:], AF.Ln, bias=1.0)
                    al_l.append(alpha)
                for ti in range(TI):
                    alpha = al_l[ti]
                    m8 = pa_sm.tile([P, 8], F32, tag="m8")
                    nc.vector.max(out=m8[:], in_=alpha[:])
                    nc.vector.reduce_sum(sall[:, ti : ti + 1], m8[:], axis=AX.X)
                    repl = pa.tile([P, NB], F32, tag="repl")
                    nc.vector.match_replace(out=repl[:], in_to_replace=m8[:],
                                            in_values=alpha[:], imm_value=0.0)
                    nc.vector.tensor_sub(zsall[:, ti, :], alpha[:], repl[:])
                for ti in range(TI):
                    nc.scalar.activation(thall[:, ti : ti + 1],
                                         sall[:, ti : ti + 1], AF.Tanh)
                for ti in range(TI):
                    sp = pa_sm.tile([P, 1], F32, tag="sp")
                    nc.vector.tensor_scalar_add(sp[:], sall[:, ti : ti + 1], EPS)
                    nc.vector.reciprocal(sp[:], sp[:])
                    q = pa_sm.tile([P, 1], F32, tag="q")
                    nc.vector.tensor_tensor(q[:], thall[:, ti : ti + 1], sp[:],
                                            OP.mult)
                    nc.vector.scalar_tensor_tensor(
                        gall[:, ti, :], zsall[:, ti, :], q[:], hfall[:, ti, :],
                        OP.mult, OP.mult)

        # ============ B/C: FFN + output, token-halved ============
        with tc.tile_pool(name="bigw", bufs=2) as bigw, \
             tc.tile_pool(name="pw2", bufs=3) as pw2, \
             tc.tile_pool(name="bigp", bufs=1) as bigp, \
             tc.tile_pool(name="pb", bufs=6) as pb, \
             tc.tile_pool(name="ppt", bufs=3, space="PSUM") as ppt:

            def ffn1_half(half, ppb, emit_t=None):
                hsl = slice(half * 512, (half + 1) * 512)
                gh = bigp.tile([P, HJ, 512], BF16, tag="gh")
                for hj in range(HJ):
                    if emit_t is not None and 16 <= hj < 24:
                        emit_t(hj - 16)
                    w1f = pb.tile([P, DK, P], F32, tag="w1f")
                    nc.sync.dma_start(w1f[:], w1t_v[:, :, hj * P : (hj + 1) * P])
                    w1c = pb.tile([P, DK, P], BF16, tag="w1c")
                    if hj % 2 == 0:
                        nc.scalar.copy(
                            w1c[:].rearrange("p a b -> p (a b)"),
                            w1f[:].rearrange("p a b -> p (a b)"))
                    else:
                        nc.gpsimd.tensor_copy(
                            w1c[:].rearrange("p a b -> p (a b)"),
                            w1f[:].rearrange("p a b -> p (a b)"))
                    hps = ppb.tile([P, 512], F32, tag="hps")
                    for dk in range(DK):
                        nc.tensor.matmul(hps[:], w1c[:, dk, :], xtb[:, dk, hsl],
                                         start=(dk == 0), stop=(dk == DK - 1))
                    nc.scalar.activation(gh[:, hj, :], hps[:], GELU)
                return gh

            def out_half(half, gh, pc, ppc):
                for dh in range(2):
                    dsl = slice(dh * 512, (dh + 1) * 512)
                    w2h = bigw.tile([P, HJ, 512], BF16, tag="w2h")
                    for ch in range(HJ // 2):
                        w2f = pw2.tile([P, 2, 512], F32, tag="w2f")
                        nc.sync.dma_start(
                            w2f[:], w2t_v[:, ch * 2 : (ch + 1) * 2, dsl])
                        nc.vector.tensor_copy(
                            w2h[:, ch * 2 : (ch + 1) * 2, :].rearrange(
                                "p a b -> p (a b)"),
                            w2f[:].rearrange("p a b -> p (a b)"))
                    for ti4 in range(4):
                        ti = half * 4 + ti4
                        tsl = slice(ti * P, (ti + 1) * P)
                        t4sl = slice(ti4 * P, (ti4 + 1) * P)
                        ops = ppc.tile([P, 512], F32, tag="ops")
                        for hj in range(HJ):
                            nc.tensor.matmul(ops[:], gh[:, hj, t4sl],
                                             w2h[:, hj, :],
                                             start=(hj == 0), stop=False)
                        for nbj in range(NBJ):
                            nc.tensor.matmul(ops[:], gt[:, nbj, tsl],
                                             vs[:, nbj, dsl],
                                             start=False, stop=(nbj == NBJ - 1))
                        o_sb = pc.tile([P, 512], F32, tag="o_sb")
                        nc.vector.tensor_copy(o_sb[:], ops[:])
                        nc.sync.dma_start(out_v[:, ti, dsl], o_sb[:])

            def emit_transpose(ti):
                tsl = slice(ti * P, (ti + 1) * P)
                for nbj in range(NBJ):
                    pt = ppt.tile([P, P], BF16, tag="pt")
                    nc.tensor.transpose(
                        pt[:], gall[:, ti, nbj * P : (nbj + 1) * P], ident[:])
                    nc.vector.tensor_copy(gt[:, nbj, tsl], pt[:])

            with tc.tile_pool(name="ppb0", bufs=3, space="PSUM") as ppb0:
                gh0 = ffn1_half(0, ppb0, emit_t=emit_transpose)

            with tc.tile_pool(name="pc", bufs=2) as pc, \
                 tc.tile_pool(name="ppc", bufs=3, space="PSUM") as ppc, \
                 tc.tile_pool(name="ppb1", bufs=2, space="PSUM") as ppb1:
                out_half(0, gh0, pc, ppc)
                gh1 = ffn1_half(1, ppb1)
                out_half(1, gh1, pc, ppc)

    nc.compile()
    return nc


_cached_nc = None
_EYE = np.eye(P, dtype=np.float32)


def kernel(x, W1, W2, ln_g, ln_b, router_W, router_b, raw_U, raw_V, gamma):
    global _cached_nc
    x = np.ascontiguousarray(np.asarray(x, np.float32)).reshape(-1, D)
    w1t = np.ascontiguousarray(np.asarray(W1, np.float32).T)
    w2t = np.ascontiguousarray(np.asarray(W2, np.float32).T)
    rwt = np.ascontiguousarray(np.asarray(router_W, np.float32).T)
    utt = np.ascontiguousarray(np.asarray(raw_U, np.float32).T)
    vv = np.ascontiguousarray(np.asarray(raw_V, np.float32))
    gcol = np.ascontiguousarray(np.asarray(ln_g, np.float32).reshape(DK, P).T)
    bcol = np.ascontiguousarray(np.asarray(ln_b, np.float32).reshape(DK, P).T)
    rb = np.ascontiguousarray(np.asarray(router_b, np.float32).reshape(1, NB))
    gam = np.ascontiguousarray(np.asarray(gamma, np.float32).reshape(1, D))

    if _cached_nc is None:
        _cached_nc = _build()
    nc = _cached_nc

    in_maps = []
    for c in range(NCORE):
        shard = x[c * T : (c + 1) * T]
        in_maps.append({
            "xt": np.ascontiguousarray(shard.T),
            "w1t": w1t, "w2t": w2t, "rwt": rwt, "ut": utt, "v": vv,
            "gcol": gcol, "bcol": bcol, "rb": rb, "gam": gam,
            "eye": _EYE,
        })
    res = run_bass_kernel_spmd(nc, in_maps, list(range(NCORE)))
    kernel._last_results = res
    out = np.concatenate([res.results[c]["out"] for c in range(NCORE)], axis=0)
    return out.reshape(4, 2048, D)



# revision 7
# speedup vs baseline: 1.3488x; 1.3488x over previous
"""DSC layer (moe_routing) on 8 TRN2 NeuronCores, data-parallel over tokens.

fp8 DoubleRow formulation. All big matmuls run as fp8e4 (e4m3) DoubleRow
pairs (two 128-row k-tiles per PE instruction at 0.5 cycles per output
row) with a hi+lo error-compensation split on the precision-critical
FFN path:

  x   ~= (x_hi + x_lo)/4            (two e4m3 planes, scale 4)
  W1  ~= (W1_hi + W1_lo)/32
  h    = (x_hi+x_lo)@W1_hi [dup-pair]  +  x_hi@W1_lo [tile-pair]
  gh   = gelu(h)   (ACT, f32) -> gh_hi = fp8(gh), gh_lo = fp8(gh - gh_hi)
  W2  ~= (W2_hi + W2_lo)/256
  out  = (gh_hi+gh_lo)@W2_hi + gh_hi@W2_lo + dyn      (PSUM accum, /256)

The dyn path (router top-8 + U/V basis) contributes ~0.17% of the output
norm, so it runs in pure fp8: router logits r0 = x_hi@wg' with the LN
mean-correction folded into wg' (wg' = g*rW - colsum/D), h_full =
x_hi@U_norm, dyn = G@(V_norm*gamma). G is transposed on PE in bf16 and
cast to fp8 at the PSUM evict. LN stats (mu, var) come from [t,1]
matmuls against a ones column (stationary = x bf16).

U/V normalization, router weight folding, transposed layouts, and the
fp8 hi/lo weight splits are host-side prep; only math runs on device.
"""
import sys, os
sys.path.insert(0, "/opt/trn_rl_repo")
from contextlib import ExitStack
import numpy as np
import ml_dtypes
import concourse.bass as bass
import concourse.mybir as mybir
from concourse import bacc
from concourse.tile import TileContext
from concourse.bass_utils import run_bass_kernel_spmd

F32 = mybir.dt.float32
BF16 = mybir.dt.bfloat16
FP8 = mybir.dt.float8e4
AF = mybir.ActivationFunctionType
OP = mybir.AluOpType
DR = mybir.MatmulPerfMode.DoubleRow
FP8NP = ml_dtypes.float8_e4m3
BF16NP = ml_dtypes.bfloat16

D, NB, H = 1024, 512, 4096
NCORE = 8
T = 1024          # tokens per core
P = 128
TI = T // P       # 8 token tiles
DK = D // P       # 8 k-tiles over D
HJ = H // P       # 32 tiles over ffn hidden
NBJ = NB // P     # 4 tiles over basis dim
TAU = 10.0
EPS = 1e-6

SX = 4.0          # x fp8 scale
SW1 = 32.0        # W1 fp8 scale
SWG = 256.0       # router weight fp8 scale
SU = 32.0         # U_norm fp8 scale
SVG = 64.0        # (V_norm*gamma) fp8 scale
SW2 = 256.0       # W2 fp8 scale
GELU_SC = 1.0 / (SX * SW1)                  # FFN1 psum -> true h
RS_BIAS = float(np.log(1.0 / (SX * SWG)))   # fold router psum scale into rs
QF = 4.0 / (SX * SU)                        # fold hf psum scale + G fp8 scale
OUT_SC = 1.0 / SW2                          # FFN2 psum -> true out


def _build():
    nc = bacc.Bacc("TRN2", target_bir_lowering=False, debug=False, num_devices=NCORE)
    x16_e = nc.declare_dram_parameter("x16", [D, T], BF16, isOutput=False)
    xhl_e = nc.declare_dram_parameter("xhl", [2 * D, T], FP8, isOutput=False)
    w1_e = nc.declare_dram_parameter("w1", [P * HJ, 2 * DK * P], FP8, isOutput=False)
    w2_e = nc.declare_dram_parameter("w2", [P * 4, 2 * HJ * 256], FP8, isOutput=False)
    wg_e = nc.declare_dram_parameter("wg", [D, NB], FP8, isOutput=False)
    un_e = nc.declare_dram_parameter("un", [D, NB], FP8, isOutput=False)
    vg_e = nc.declare_dram_parameter("vg", [NB, D], FP8, isOutput=False)
    c16_e = nc.declare_dram_parameter("c16", [1, NB], BF16, isOutput=False)
    eye_e = nc.declare_dram_parameter("eye", [P, P], BF16, isOutput=False)
    out_e = nc.declare_dram_parameter("out", [T, D], F32, isOutput=True)

    x16_v = x16_e[:].rearrange("(k p) t -> p k t", p=P)       # [128, 8, T]
    xhl_v = xhl_e[:].rearrange("(k p) t -> p k t", p=P)       # [128, 16, T]
    w1_v = w1_e[:].rearrange("(p h) x -> p h x", p=P)         # [128, 32, 2048]
    w2_v = w2_e[:].rearrange("(p c) x -> p c x", p=P)         # [128, 4, 16384]
    wg_v = wg_e[:].rearrange("(k p) n -> p k n", p=P)
    un_v = un_e[:].rearrange("(k p) n -> p k n", p=P)
    vg_v = vg_e[:].rearrange("(j p) d -> p j d", p=P)
    out_v = out_e[:].rearrange("(to p) d -> p to d", p=P)

    with TileContext(nc) as tc, ExitStack() as ctx:
        const = ctx.enter_context(tc.tile_pool(name="const", bufs=1))
        persist = ctx.enter_context(tc.tile_pool(name="persist", bufs=1))
        w2p = ctx.enter_context(tc.tile_pool(name="w2p", bufs=2))

        ones_col = const.tile([P, 1], BF16)
        nc.vector.memset(ones_col[:], 1.0)
        ones_row = const.tile([1, P], BF16)
        nc.vector.memset(ones_row[:], 1.0)
        epsb = const.tile([P, 1], F32)
        nc.vector.memset(epsb[:], 1e-5)
        rsbias = const.tile([P, 1], F32)
        nc.vector.memset(rsbias[:], RS_BIAS)
        ident = const.tile([P, P], BF16)
        c_b = const.tile([P, NB], F32)

        xhl = persist.tile([P, 2 * DK, T], FP8)    # 16K/part
        vg = persist.tile([P, NBJ, D], FP8)        # 4K
        gt = persist.tile([P, NBJ, T], FP8)        # 4K (fp8(4*G^T))
        hfall = persist.tile([P, TI, NB], BF16)    # 8K (128*h_lat)
        zsall = persist.tile([P, TI, NB], BF16)    # 8K (masked alpha)
        gall = persist.tile([P, TI, NB], BF16)     # 8K (4*G pre-transpose)
        ghHL = persist.tile([P, HJ, 2, T], FP8)    # 64K (gelu hi/lo planes)
        rs_t = persist.tile([P, TI], F32)
        sall = persist.tile([P, TI], F32)
        thall = persist.tile([P, TI], F32)
        qall = persist.tile([P, TI], F32)

        # xhl planes as [parity, k]: xv2[:, 0, j] = x_hi tile j
        xv2 = xhl[:].rearrange("p (k two) t -> p two k t", two=2)

        # ---------------- stats scope ----------------
        with tc.tile_pool(name="pst", bufs=1) as pst, \
             tc.tile_pool(name="psS", bufs=2, space="PSUM") as psS:
            x16 = pst.tile([P, DK, T], BF16, tag="x16")
            for dk in range(DK):
                nc.sync.dma_start(x16[:, dk, :], x16_v[:, dk, :])
            xsq = pst.tile([P, DK, T], BF16, tag="xsq")
            for dk in range(DK):
                nc.vector.tensor_tensor(xsq[:, dk, :], x16[:, dk, :],
                                        x16[:, dk, :], OP.mult)
            musq = pst.tile([P, TI, 2], F32, tag="musq")
            for ti in range(TI):
                tsl = slice(ti * P, (ti + 1) * P)
                ps = psS.tile([P, 2], F32, tag="pstat")
                for dk in range(DK):
                    nc.tensor.matmul(ps[:, 0:1], x16[:, dk, tsl], ones_col[:],
                                     start=(dk == 0), stop=False,
                                     skip_group_check=True)
                for dk in range(DK):
                    nc.tensor.matmul(ps[:, 1:2], xsq[:, dk, tsl], ones_col[:],
                                     start=False, stop=(dk == DK - 1),
                                     skip_group_check=True)
                nc.vector.tensor_copy(musq[:, ti, :], ps[:])
            mu_all = pst.tile([P, TI], F32, tag="mu_all")
            sq_all = pst.tile([P, TI], F32, tag="sq_all")
            var_all = pst.tile([P, TI], F32, tag="var_all")
            nc.vector.tensor_scalar_mul(mu_all[:], musq[:, :, 0], 1.0 / D)
            nc.vector.tensor_scalar_mul(sq_all[:], musq[:, :, 1], 1.0 / D)
            nc.vector.tensor_tensor(var_all[:], mu_all[:], mu_all[:], OP.mult)
            nc.vector.tensor_sub(var_all[:], sq_all[:], var_all[:])
            lnv = pst.tile([P, TI], F32, tag="lnv")
            nc.scalar.activation(lnv[:], var_all[:], AF.Ln, bias=epsb[:])
            nc.scalar.activation(rs_t[:], lnv[:], AF.Exp, scale=-0.5,
                                 bias=rsbias[:])

        # ---------------- A-phase + FFN1 scope ----------------
        with tc.tile_pool(name="tabs", bufs=1) as tabs, \
             tc.tile_pool(name="w1p", bufs=5) as w1p, \
             tc.tile_pool(name="pa", bufs=2) as pa, \
             tc.tile_pool(name="pasm", bufs=3) as pasm, \
             tc.tile_pool(name="pgh", bufs=3) as pgh, \
             tc.tile_pool(name="psA", bufs=1, space="PSUM") as psA, \
             tc.tile_pool(name="psF", bufs=3, space="PSUM") as psF, \
             tc.tile_pool(name="psT", bufs=2, space="PSUM") as psT:

            # DMA queue (SP) order: xhl -> w1[0..3] -> wg/un/c16/eye -> w1 rest
            nc.sync.dma_start(xhl[:, 0:DK, :], xhl_v[:, 0:DK, :])
            nc.sync.dma_start(xhl[:, DK:2 * DK, :], xhl_v[:, DK:2 * DK, :])
            w1tiles = []
            for hj in range(4):
                w1b = w1p.tile([P, 2, DK, P], FP8, tag="w1b")
                nc.sync.dma_start(
                    w1b[:].rearrange("p a b c -> p (a b c)"), w1_v[:, hj, :])
                w1tiles.append(w1b)
            wg = tabs.tile([P, DK, NB], FP8)
            un = tabs.tile([P, DK, NB], FP8)
            nc.sync.dma_start(wg[:], wg_v[:])
            nc.sync.dma_start(un[:], un_v[:])
            c16 = tabs.tile([1, NB], BF16)
            nc.sync.dma_start(c16[:], c16_e[:])
            eyef = tabs.tile([P, P], BF16, tag="eyef")
            nc.sync.dma_start(eyef[:], eye_e[:])
            nc.gpsimd.tensor_copy(ident[:], eyef[:])
            nc.sync.dma_start(vg[:], vg_v[:])
            # c_b broadcast
            cps = psA.tile([P, NB], F32, tag="pcb")
            nc.tensor.matmul(cps[:], ones_row[:], c16[:], start=True, stop=True)
            nc.vector.tensor_copy(c_b[:], cps[:])
            # W2 chunks on the ACT queue (own pacing, no head-of-line with w1)
            w2tiles = []
            for c in range(4):
                w2b = w2p.tile([P, 2, HJ, 256], FP8, tag="w2b")
                nc.scalar.dma_start(
                    w2b[:].rearrange("p a b c -> p (a b c)"), w2_v[:, c, :])
                w2tiles.append(w2b)

            def emit_A(ti):
                """Router + h_full + softplus/top-8/G for one token tile."""
                tsl = slice(ti * P, (ti + 1) * P)
                r0 = psA.tile([P, NB], F32, tag="pArt")
                for nbc in range(2):
                    nsl = slice(nbc * 256, (nbc + 1) * 256)
                    for j in range(4):
                        nc.tensor.matmul(
                            r0[:, nsl], xv2[:, 0, 2 * j:2 * j + 2, tsl],
                            wg[:, 2 * j:2 * j + 2, nsl],
                            start=(nbc == 0 and j == 0),
                            stop=(nbc == 1 and j == 3),
                            perf_mode=DR, skip_group_check=True)
                rf = pa.tile([P, NB], F32, tag="rf")
                nc.vector.scalar_tensor_tensor(
                    rf[:], r0[:], rs_t[:, ti:ti + 1], c_b[:], OP.mult, OP.add)
                nc.gpsimd.tensor_scalar(rf[:], rf[:], TAU, -TAU, OP.min, OP.max)
                hf = psA.tile([P, NB], F32, tag="pAhf")
                for nbc in range(2):
                    nsl = slice(nbc * 256, (nbc + 1) * 256)
                    for j in range(4):
                        nc.tensor.matmul(
                            hf[:, nsl], xv2[:, 0, 2 * j:2 * j + 2, tsl],
                            un[:, 2 * j:2 * j + 2, nsl],
                            start=(nbc == 0 and j == 0),
                            stop=(nbc == 1 and j == 3),
                            perf_mode=DR, skip_group_check=True)
                nc.scalar.copy(hfall[:, ti, :], hf[:])
                e_sb = pa.tile([P, NB], F32, tag="e_sb")
                nc.scalar.activation(e_sb[:], rf[:], AF.Exp)
                alpha = pa.tile([P, NB], F32, tag="alpha")
                nc.scalar.activation(alpha[:], e_sb[:], AF.Ln, bias=1.0)
                m8 = pasm.tile([P, 8], F32, tag="m8")
                nc.vector.max(out=m8[:], in_=alpha[:])
                nc.vector.reduce_sum(sall[:, ti:ti + 1], m8[:],
                                     axis=mybir.AxisListType.X)
                repl = pa.tile([P, NB], F32, tag="repl")
                nc.vector.match_replace(out=repl[:], in_to_replace=m8[:],
                                        in_values=alpha[:], imm_value=0.0)
                nc.gpsimd.tensor_tensor(zsall[:, ti, :], alpha[:], repl[:],
                                        OP.subtract)
                nc.scalar.activation(thall[:, ti:ti + 1], sall[:, ti:ti + 1],
                                     AF.Tanh)
                sp = pasm.tile([P, 1], F32, tag="sp")
                nc.vector.tensor_scalar_add(sp[:], sall[:, ti:ti + 1], EPS)
                nc.vector.reciprocal(sp[:], sp[:])
                nc.vector.scalar_tensor_tensor(
                    qall[:, ti:ti + 1], thall[:, ti:ti + 1], QF, sp[:],
                    OP.mult, OP.mult)
                nc.vector.scalar_tensor_tensor(
                    gall[:, ti, :], zsall[:, ti, :], qall[:, ti:ti + 1],
                    hfall[:, ti, :], OP.mult, OP.mult)

            def emit_T(ti):
                tsl = slice(ti * P, (ti + 1) * P)
                for nbj in range(NBJ):
                    pt = psT.tile([P, P], BF16, tag="pt")
                    nc.tensor.transpose(
                        pt[:], gall[:, ti, nbj * P:(nbj + 1) * P], ident[:])
                    nc.vector.tensor_copy(gt[:, nbj, tsl], pt[:])

            for hj in range(HJ):
                if hj + 4 < HJ:
                    w1b = w1p.tile([P, 2, DK, P], FP8, tag="w1b")
                    nc.sync.dma_start(
                        w1b[:].rearrange("p a b c -> p (a b c)"),
                        w1_v[:, hj + 4, :])
                    w1tiles.append(w1b)
                if 3 <= hj < 11:
                    emit_A(hj - 3)
                if 13 <= hj < 21:
                    emit_T(hj - 13)
                w1b = w1tiles[hj]
                for half in range(2):
                    hsl = slice(half * 512, (half + 1) * 512)
                    ps = psF.tile([P, 512], F32, tag="pF1")
                    for j in range(DK):
                        nc.tensor.matmul(
                            ps[:], w1b[:, 0:1, j, :].to_broadcast([P, 2, P]),
                            xhl[:, 2 * j:2 * j + 2, hsl],
                            start=(j == 0), stop=False,
                            perf_mode=DR, skip_group_check=True)
                    for j in range(4):
                        nc.tensor.matmul(
                            ps[:], w1b[:, 1, 2 * j:2 * j + 2, :],
                            xv2[:, 0, 2 * j:2 * j + 2, hsl],
                            start=False, stop=(j == 3),
                            perf_mode=DR, skip_group_check=True)
                    gh16 = pgh.tile([P, 512], F32, tag="gh16")
                    nc.scalar.activation(gh16[:], ps[:], AF.Gelu, scale=GELU_SC)
                    nc.gpsimd.tensor_copy(ghHL[:, hj, 0, hsl], gh16[:])
                    nc.vector.scalar_tensor_tensor(
                        ghHL[:, hj, 1, hsl], gh16[:], 1.0, ghHL[:, hj, 0, hsl],
                        OP.mult, OP.subtract)

        # ---------------- FFN2 + dyn ----------------
        with tc.tile_pool(name="po", bufs=3) as po, \
             tc.tile_pool(name="psO", bufs=4, space="PSUM") as psO:
            for c in range(4):
                csl = slice(c * 256, (c + 1) * 256)
                w2b = w2tiles[c]
                for ti in range(TI):
                    tsl = slice(ti * P, (ti + 1) * P)
                    ps = psO.tile([P, 256], F32, tag="pO")
                    for hj in range(HJ):
                        nc.tensor.matmul(
                            ps[:], ghHL[:, hj, :, tsl],
                            w2b[:, 0:1, hj, :].to_broadcast([P, 2, 256]),
                            start=(hj == 0), stop=False,
                            perf_mode=DR, skip_group_check=True)
                    for j in range(HJ // 2):
                        nc.tensor.matmul(
                            ps[:], ghHL[:, 2 * j:2 * j + 2, 0, tsl],
                            w2b[:, 1, 2 * j:2 * j + 2, :],
                            start=False, stop=False,
                            perf_mode=DR, skip_group_check=True)
                    for j in range(NBJ // 2):
                        nc.tensor.matmul(
                            ps[:], gt[:, 2 * j:2 * j + 2, tsl],
                            vg[:, 2 * j:2 * j + 2, csl],
                            start=False, stop=(j == NBJ // 2 - 1),
                            perf_mode=DR, skip_group_check=True)
                    o_sb = po.tile([P, 256], F32, tag="o_sb")
                    nc.scalar.mul(o_sb[:], ps[:], OUT_SC)
                    nc.gpsimd.dma_start(out_v[:, ti, csl], o_sb[:])

    nc.compile()
    return nc


_cached_nc = None


def _fp8_split(a, scale):
    hi = (a * scale).astype(FP8NP)
    lo = (a * scale - hi.astype(np.float32)).astype(FP8NP)
    return hi, lo


def _prep_weights(W1, W2, ln_g, ln_b, router_W, router_b, raw_U, raw_V, gamma):
    W1 = np.asarray(W1, np.float32)
    W2 = np.asarray(W2, np.float32)
    ln_g = np.asarray(ln_g, np.float32)
    ln_b = np.asarray(ln_b, np.float32)
    router_W = np.asarray(router_W, np.float32)
    router_b = np.asarray(router_b, np.float32)
    raw_U = np.asarray(raw_U, np.float32)
    raw_V = np.asarray(raw_V, np.float32)
    gam = np.asarray(gamma, np.float32).reshape(D)

    # w1: [(p hj), (two k c)]
    w1hi, w1lo = _fp8_split(W1.T, SW1)                        # [D, H]
    w1s = np.stack([w1hi, w1lo], 0).reshape(2, DK, P, HJ, P)  # 2 k p hj c
    w1s = np.ascontiguousarray(np.transpose(w1s, (2, 3, 0, 1, 4)))
    w1s = w1s.reshape(P * HJ, 2 * DK * P)

    # w2: chunk-major [(p c4), (two hj 256)]
    w2hi, w2lo = _fp8_split(W2.T, SW2)                        # [H, D]
    w2s = np.stack([w2hi, w2lo], 0).reshape(2, HJ, P, 4, 256)  # 2 hj p c d
    w2s = np.ascontiguousarray(np.transpose(w2s, (2, 3, 0, 1, 4)))
    w2s = w2s.reshape(P * 4, 2 * HJ * 256)

    wgm = (router_W * ln_g[None, :]).T                        # [D, NB]
    sg = wgm.sum(axis=0)
    wgp = np.ascontiguousarray(((wgm - sg[None, :] / D) * SWG).astype(FP8NP))
    cvec = ln_b @ router_W.T + router_b
    c16 = np.ascontiguousarray(cvec.astype(BF16NP).reshape(1, NB))

    un = raw_U / np.maximum(np.linalg.norm(raw_U, axis=1, keepdims=True), EPS)
    unp = np.ascontiguousarray((un.T * SU).astype(FP8NP))      # [D, NB]
    vn = raw_V / np.maximum(np.linalg.norm(raw_V, axis=1, keepdims=True), EPS)
    vgp = np.ascontiguousarray((vn * gam[None, :] * SVG).astype(FP8NP))

    eye = np.ascontiguousarray(np.eye(P, dtype=np.float32).astype(BF16NP))
    return {
        "w1": w1s, "w2": w2s, "wg": wgp, "un": unp, "vg": vgp,
        "c16": c16, "eye": eye,
    }


def kernel(x, W1, W2, ln_g, ln_b, router_W, router_b, raw_U, raw_V, gamma):
    global _cached_nc
    x = np.asarray(x, np.float32).reshape(-1, D)

    if _cached_nc is None:
        _cached_nc = _build()
    nc = _cached_nc
    wmap = _prep_weights(W1, W2, ln_g, ln_b, router_W, router_b,
                         raw_U, raw_V, gamma)

    in_maps = []
    for cidx in range(NCORE):
        shard = x[cidx * T:(cidx + 1) * T]                 # [T, D]
        xt = np.ascontiguousarray(shard.T)                 # [D, T]
        x16c = xt.astype(BF16NP)
        xhi = (xt * SX).astype(FP8NP)
        xlo = (xt * SX - xhi.astype(np.float32)).astype(FP8NP)
        xhl = np.empty((2 * DK, P, T), FP8NP)
        xhl[0::2] = xhi.reshape(DK, P, T)
        xhl[1::2] = xlo.reshape(DK, P, T)
        in_maps.append({
            "x16": x16c, "xhl": np.ascontiguousarray(xhl.reshape(2 * D, T)),
            **wmap,
        })
    res = run_bass_kernel_spmd(nc, in_maps, list(range(NCORE)))
    kernel._last_results = res
    out = np.concatenate([res.results[c]["out"] for c in range(NCORE)], axis=0)
    return out.reshape(4, 2048, D)


# revision 12
# speedup vs baseline: 1.4104x; 1.0457x over previous
"""DSC layer (moe_routing) on 8 TRN2 NeuronCores, data-parallel over tokens.

fp8 DoubleRow formulation. All big matmuls run as fp8e4 (e4m3) DoubleRow
pairs (two 128-row k-tiles per PE instruction at 0.5 cycles per output
row) with a hi+lo error-compensation split on the precision-critical
FFN path:

  x   ~= (x_hi + x_lo)/4            (two e4m3 planes, scale 4)
  W1  ~= (W1_hi + W1_lo)/32
  h    = (x_hi+x_lo)@W1_hi [dup-pair]  +  x_hi@W1_lo [tile-pair]
  gh   = gelu(h)   (ACT, f32) -> gh_hi = fp8(gh), gh_lo = fp8(gh - gh_hi)
  W2  ~= (W2_hi + W2_lo)/256
  out  = (gh_hi+gh_lo)@W2_hi + gh_hi@W2_lo + dyn      (PSUM accum, /256)

The dyn path (router top-8 + U/V basis) contributes ~0.17% of the output
norm, so it runs in pure fp8: router logits r0 = x_hi@wg' with the LN
mean-correction folded into wg' (wg' = g*rW - colsum/D), h_full =
x_hi@U_norm, dyn = G@(V_norm*gamma). G is transposed on PE in bf16 and
cast to fp8 at the PSUM evict. LN stats (mu, var) come from [t,1]
matmuls against a ones column (stationary = x bf16, squared in place for
the second moment). tanh(S) is computed as 1 - 2/(exp(2S)+1) so that the
A-phase only ever uses the {exp, ln} activation-table set; all ACT
table switches are batched (2 per token-tile pair instead of ~4).

U/V normalization, router weight folding, transposed layouts, and the
fp8 hi/lo weight splits are host-side prep; only math runs on device.
"""
import sys, os
sys.path.insert(0, "/opt/trn_rl_repo")
from contextlib import ExitStack
import numpy as np
import ml_dtypes
import concourse.bass as bass
import concourse.mybir as mybir
from concourse import bacc
from concourse.tile import TileContext
from concourse.bass_utils import run_bass_kernel_spmd

F32 = mybir.dt.float32
BF16 = mybir.dt.bfloat16
FP8 = mybir.dt.float8e4
AF = mybir.ActivationFunctionType
OP = mybir.AluOpType
DR = mybir.MatmulPerfMode.DoubleRow
FP8NP = ml_dtypes.float8_e4m3
BF16NP = ml_dtypes.bfloat16

D, NB, H = 1024, 512, 4096
NCORE = 8
T = 1024          # tokens per core
P = 128
TI = T // P       # 8 token tiles
DK = D // P       # 8 k-tiles over D
HJ = H // P       # 32 tiles over ffn hidden
NBJ = NB // P     # 4 tiles over basis dim
TAU = 10.0
EPS = 1e-6

SX = 4.0          # x fp8 scale
SW1 = 32.0        # W1 fp8 scale
SWG = 256.0       # router weight fp8 scale
SU = 32.0         # U_norm fp8 scale
SVG = 64.0        # (V_norm*gamma) fp8 scale
SW2 = 256.0       # W2 fp8 scale
GELU_SC = 1.0 / (SX * SW1)                  # FFN1 psum -> true h
RS_BIAS = float(np.log(1.0 / (SX * SWG)))   # fold router psum scale into rs
QF = 4.0 / (SX * SU)                        # fold hf psum scale + G fp8 scale
OUT_SC = 1.0 / SW2                          # FFN2 psum -> true out


def _build():
    nc = bacc.Bacc("TRN2", target_bir_lowering=False, debug=False, num_devices=NCORE)
    x16_e = nc.declare_dram_parameter("x16", [D, T], BF16, isOutput=False)
    xhl_e = nc.declare_dram_parameter("xhl", [2 * D, T], FP8, isOutput=False)
    w1_e = nc.declare_dram_parameter("w1", [P * HJ, 2 * DK * P], FP8, isOutput=False)
    w2_e = nc.declare_dram_parameter("w2", [P * 4, 2 * HJ * 256], FP8, isOutput=False)
    wg_e = nc.declare_dram_parameter("wg", [D, NB], FP8, isOutput=False)
    un_e = nc.declare_dram_parameter("un", [D, NB], FP8, isOutput=False)
    vg_e = nc.declare_dram_parameter("vg", [NB, D], FP8, isOutput=False)
    c16_e = nc.declare_dram_parameter("c16", [1, NB], BF16, isOutput=False)
    eye_e = nc.declare_dram_parameter("eye", [P, P], BF16, isOutput=False)
    out_e = nc.declare_dram_parameter("out", [T, D], F32, isOutput=True)

    x16_v = x16_e[:].rearrange("(k p) t -> p k t", p=P)       # [128, 8, T]
    xhl_v = xhl_e[:].rearrange("(k p) t -> p k t", p=P)       # [128, 16, T]
    w1_v = w1_e[:].rearrange("(p h) x -> p h x", p=P)         # [128, 32, 2048]
    w2_v = w2_e[:].rearrange("(p c) x -> p c x", p=P)         # [128, 4, 16384]
    wg_v = wg_e[:].rearrange("(k p) n -> p k n", p=P)
    un_v = un_e[:].rearrange("(k p) n -> p k n", p=P)
    vg_v = vg_e[:].rearrange("(j p) d -> p j d", p=P)
    out_v = out_e[:].rearrange("(to p) d -> p to d", p=P)

    with TileContext(nc) as tc, ExitStack() as ctx:
        const = ctx.enter_context(tc.tile_pool(name="const", bufs=1))
        persist = ctx.enter_context(tc.tile_pool(name="persist", bufs=1))
        w2p = ctx.enter_context(tc.tile_pool(name="w2p", bufs=2))

        ones_col = const.tile([P, 1], BF16)
        nc.vector.memset(ones_col[:], 1.0)
        ones_row = const.tile([1, P], BF16)
        nc.vector.memset(ones_row[:], 1.0)
        epsb = const.tile([P, 1], F32)
        nc.vector.memset(epsb[:], 1e-5)
        rsbias = const.tile([P, 1], F32)
        nc.vector.memset(rsbias[:], RS_BIAS)
        ident = const.tile([P, P], BF16)
        c_b = const.tile([P, NB], F32)

        xhl = persist.tile([P, 2 * DK, T], FP8)    # 16K/part
        vg = persist.tile([P, NBJ, D], FP8)        # 4K
        gt = persist.tile([P, NBJ, T], FP8)        # 4K (fp8(4*G^T))
        hfall = persist.tile([P, TI, NB], BF16)    # 8K (128*h_lat)
        zsall = persist.tile([P, TI, NB], BF16)    # 8K (masked alpha)
        gall = persist.tile([P, TI, NB], BF16)     # 8K (4*G pre-transpose)
        ghHL = persist.tile([P, HJ, 2, T], FP8)    # 64K (gelu hi/lo planes)
        rs_t = persist.tile([P, TI], F32)
        sall = persist.tile([P, TI], F32)
        thall = persist.tile([P, TI], F32)
        qall = persist.tile([P, TI], F32)

        # xhl planes as [parity, k]: xv2[:, 0, j] = x_hi tile j
        xv2 = xhl[:].rearrange("p (k two) t -> p two k t", two=2)

        ctx2 = ExitStack()
        tabs = ctx2.enter_context(tc.tile_pool(name="tabs", bufs=1))
        w1p = ctx2.enter_context(tc.tile_pool(name="w1p", bufs=5))
        pa = ctx2.enter_context(tc.tile_pool(name="pa", bufs=2))
        pasm = ctx2.enter_context(tc.tile_pool(name="pasm", bufs=3))
        pgh = ctx2.enter_context(tc.tile_pool(name="pgh", bufs=3))
        psF = ctx2.enter_context(tc.tile_pool(name="psF", bufs=3, space="PSUM"))

        # ---------- DMA queue (SP) order: xhl -> w1[0..3] -> x16 -> tables
        nc.sync.dma_start(xhl[:, 0:DK, :], xhl_v[:, 0:DK, :])
        nc.sync.dma_start(xhl[:, DK:2 * DK, :], xhl_v[:, DK:2 * DK, :])
        w1tiles = []
        for hj in range(4):
            w1b = w1p.tile([P, 2, DK, P], FP8, tag="w1b")
            nc.sync.dma_start(
                w1b[:].rearrange("p a b c -> p (a b c)"), w1_v[:, hj, :])
            w1tiles.append(w1b)

        def ffn1_hj(hj):
            if hj + 4 < HJ:
                w1b = w1p.tile([P, 2, DK, P], FP8, tag="w1b")
                nc.sync.dma_start(
                    w1b[:].rearrange("p a b c -> p (a b c)"),
                    w1_v[:, hj + 4, :])
                w1tiles.append(w1b)
            w1b = w1tiles[hj]
            for half in range(2):
                hsl = slice(half * 512, (half + 1) * 512)
                ps = psF.tile([P, 512], F32, tag="pF1")
                for j in range(DK):
                    nc.tensor.matmul(
                        ps[:], w1b[:, 0:1, j, :].to_broadcast([P, 2, P]),
                        xhl[:, 2 * j:2 * j + 2, hsl],
                        start=(j == 0), stop=False,
                        perf_mode=DR, skip_group_check=True)
                for j in range(4):
                    nc.tensor.matmul(
                        ps[:], w1b[:, 1, 2 * j:2 * j + 2, :],
                        xv2[:, 0, 2 * j:2 * j + 2, hsl],
                        start=False, stop=(j == 3),
                        perf_mode=DR, skip_group_check=True)
                gh16 = pgh.tile([P, 512], F32, tag="gh16")
                nc.scalar.activation(gh16[:], ps[:], AF.Gelu, scale=GELU_SC)
                nc.gpsimd.tensor_copy(ghHL[:, hj, 0, hsl], gh16[:])
                nc.vector.scalar_tensor_tensor(
                    ghHL[:, hj, 1, hsl], gh16[:], 1.0, ghHL[:, hj, 0, hsl],
                    OP.mult, OP.subtract)

        # ---------- stats scope (closes before A-phase psum pools open)
        with tc.tile_pool(name="pst", bufs=1) as pst, \
             tc.tile_pool(name="psS", bufs=2, space="PSUM") as psS:
            x16 = pst.tile([P, DK, T], BF16, tag="x16")
            for dk in range(DK):
                nc.sync.dma_start(x16[:, dk, :], x16_v[:, dk, :])
            wg = tabs.tile([P, DK, NB], FP8)
            un = tabs.tile([P, DK, NB], FP8)
            nc.sync.dma_start(wg[:], wg_v[:])
            nc.sync.dma_start(un[:], un_v[:])
            c16 = tabs.tile([1, NB], BF16)
            nc.sync.dma_start(c16[:], c16_e[:])
            eyef = tabs.tile([P, P], BF16, tag="eyef")
            nc.sync.dma_start(eyef[:], eye_e[:])
            nc.gpsimd.tensor_copy(ident[:], eyef[:])
            nc.sync.dma_start(vg[:], vg_v[:])

            ffn1_hj(0)
            ffn1_hj(1)
            ffn1_hj(2)

            musq = pst.tile([P, TI, 2], F32, tag="musq")
            for ti in range(TI):
                tsl = slice(ti * P, (ti + 1) * P)
                ps = psS.tile([P, 1], F32, tag="pmu")
                for dk in range(DK):
                    nc.tensor.matmul(ps[:], x16[:, dk, tsl], ones_col[:],
                                     start=(dk == 0), stop=(dk == DK - 1))
                nc.vector.tensor_copy(musq[:, ti, 0:1], ps[:])
            for dk in range(DK):    # square in place
                nc.vector.tensor_tensor(x16[:, dk, :], x16[:, dk, :],
                                        x16[:, dk, :], OP.mult)
            for ti in range(TI):
                tsl = slice(ti * P, (ti + 1) * P)
                ps = psS.tile([P, 1], F32, tag="pmu")
                for dk in range(DK):
                    nc.tensor.matmul(ps[:], x16[:, dk, tsl], ones_col[:],
                                     start=(dk == 0), stop=(dk == DK - 1))
                nc.vector.tensor_copy(musq[:, ti, 1:2], ps[:])
            # c_b broadcast
            cps = psS.tile([P, NB], F32, tag="pcb", bufs=1)
            nc.tensor.matmul(cps[:], ones_row[:], c16[:], start=True, stop=True)
            nc.vector.tensor_copy(c_b[:], cps[:])

            mu_all = pst.tile([P, TI], F32, tag="mu_all")
            sq_all = pst.tile([P, TI], F32, tag="sq_all")
            var_all = pst.tile([P, TI], F32, tag="var_all")
            nc.vector.tensor_scalar_mul(mu_all[:], musq[:, :, 0], 1.0 / D)
            nc.vector.tensor_scalar_mul(sq_all[:], musq[:, :, 1], 1.0 / D)
            nc.vector.tensor_tensor(var_all[:], mu_all[:], mu_all[:], OP.mult)
            nc.vector.tensor_sub(var_all[:], sq_all[:], var_all[:])
            lnv = pst.tile([P, TI], F32, tag="lnv")
            nc.scalar.activation(lnv[:], var_all[:], AF.Ln, bias=epsb[:])
            nc.scalar.activation(rs_t[:], lnv[:], AF.Exp, scale=-0.5,
                                 bias=rsbias[:])

        psA = ctx2.enter_context(tc.tile_pool(name="psA", bufs=2, space="PSUM"))
        psT = ctx2.enter_context(tc.tile_pool(name="psT", bufs=1, space="PSUM"))

        def emit_A1(ti):
            """Router + h_full matmuls, logit fixup, clip (no ACT tables)."""
            tsl = slice(ti * P, (ti + 1) * P)
            r0 = psA.tile([P, NB], F32, tag="pArt")
            for nbc in range(2):
                nsl = slice(nbc * 256, (nbc + 1) * 256)
                for j in range(4):
                    nc.tensor.matmul(
                        r0[:, nsl], xv2[:, 0, 2 * j:2 * j + 2, tsl],
                        wg[:, 2 * j:2 * j + 2, nsl],
                        start=(nbc == 0 and j == 0),
                        stop=(nbc == 1 and j == 3),
                        perf_mode=DR, skip_group_check=True)
            rf = pa.tile([P, NB], F32, tag="rf")
            nc.vector.scalar_tensor_tensor(
                rf[:], r0[:], rs_t[:, ti:ti + 1], c_b[:], OP.mult, OP.add)
            nc.gpsimd.tensor_scalar(rf[:], rf[:], TAU, -TAU, OP.min, OP.max)
            hf = psA.tile([P, NB], F32, tag="pAhf")
            for nbc in range(2):
                nsl = slice(nbc * 256, (nbc + 1) * 256)
                for j in range(4):
                    nc.tensor.matmul(
                        hf[:, nsl], xv2[:, 0, 2 * j:2 * j + 2, tsl],
                        un[:, 2 * j:2 * j + 2, nsl],
                        start=(nbc == 0 and j == 0),
                        stop=(nbc == 1 and j == 3),
                        perf_mode=DR, skip_group_check=True)
            nc.vector.tensor_copy(hfall[:, ti, :], hf[:])
            return rf

        def emit_A2(ti, rf):
            """softplus pieces + top-8 + q + G. ACT stays in {exp, ln} set;
            tanh(S) = 1 - 2/(exp(2S)+1) so no table switch."""
            e_sb = pa.tile([P, NB], F32, tag="e_sb")
            nc.scalar.activation(e_sb[:], rf[:], AF.Exp)
            alpha = pa.tile([P, NB], F32, tag="alpha")
            nc.scalar.activation(alpha[:], e_sb[:], AF.Ln, bias=1.0)
            m8 = pasm.tile([P, 8], F32, tag="m8")
            nc.vector.max(out=m8[:], in_=alpha[:])
            nc.vector.reduce_sum(sall[:, ti:ti + 1], m8[:],
                                 axis=mybir.AxisListType.X)
            repl = pa.tile([P, NB], F32, tag="repl")
            nc.vector.match_replace(out=repl[:], in_to_replace=m8[:],
                                    in_values=alpha[:], imm_value=0.0)
            nc.gpsimd.tensor_tensor(zsall[:, ti, :], alpha[:], repl[:],
                                    OP.subtract)
            e2s = pasm.tile([P, 1], F32, tag="e2s")
            nc.scalar.activation(e2s[:], sall[:, ti:ti + 1], AF.Exp, scale=2.0)
            nc.vector.tensor_scalar_add(e2s[:], e2s[:], 1.0)
            nc.vector.reciprocal(e2s[:], e2s[:])
            # tanh(S) = 1 - 2*recip
            nc.vector.tensor_scalar(thall[:, ti:ti + 1], e2s[:], -2.0, 1.0,
                                    OP.mult, OP.add)
            sp = pasm.tile([P, 1], F32, tag="sp")
            nc.vector.tensor_scalar_add(sp[:], sall[:, ti:ti + 1], EPS)
            nc.vector.reciprocal(sp[:], sp[:])
            nc.vector.scalar_tensor_tensor(
                qall[:, ti:ti + 1], thall[:, ti:ti + 1], QF, sp[:],
                OP.mult, OP.mult)
            nc.vector.scalar_tensor_tensor(
                gall[:, ti, :], zsall[:, ti, :], qall[:, ti:ti + 1],
                hfall[:, ti, :], OP.mult, OP.mult)

        def emit_T(ti):
            tsl = slice(ti * P, (ti + 1) * P)
            for nbj in range(NBJ):
                pt = psT.tile([P, P], BF16, tag="pt")
                nc.tensor.transpose(
                    pt[:], gall[:, ti, nbj * P:(nbj + 1) * P], ident[:])
                nc.vector.tensor_copy(gt[:, nbj, tsl], pt[:])

        # A1 at hj {3,4, 6,7, 9,10, 12,13}; A2 pairs at hj {5, 8, 11, 14};
        # transposes at hj 16..23.
        rf_pend = {}
        w2tiles = []
        for hj in range(3, HJ):
            if hj in (3, 4, 6, 7, 9, 10, 12, 13):
                k = (3, 4, 6, 7, 9, 10, 12, 13).index(hj)
                rf_pend[k] = emit_A1(k)
            if hj in (5, 8, 11, 14):
                k = (5, 8, 11, 14).index(hj)
                emit_A2(2 * k, rf_pend.pop(2 * k))
                emit_A2(2 * k + 1, rf_pend.pop(2 * k + 1))
            if 16 <= hj < 24:
                emit_T(hj - 16)
            if hj == 10:
                for c in range(2):
                    w2b = w2p.tile([P, 2, HJ, 256], FP8, tag="w2b")
                    nc.scalar.dma_start(
                        w2b[:].rearrange("p a b c -> p (a b c)"),
                        w2_v[:, c, :])
                    w2tiles.append(w2b)
            ffn1_hj(hj)
        ctx2.close()

        # ---------------- FFN2 + dyn ----------------
        with tc.tile_pool(name="po", bufs=3) as po, \
             tc.tile_pool(name="psO", bufs=4, space="PSUM") as psO:
            for c in range(4):
                csl = slice(c * 256, (c + 1) * 256)
                w2b = w2tiles[c]
                for ti in range(TI):
                    tsl = slice(ti * P, (ti + 1) * P)
                    ps = psO.tile([P, 256], F32, tag="pO")
                    for hj in range(HJ):
                        nc.tensor.matmul(
                            ps[:], ghHL[:, hj, :, tsl],
                            w2b[:, 0:1, hj, :].to_broadcast([P, 2, 256]),
                            start=(hj == 0), stop=False,
                            perf_mode=DR, skip_group_check=True)
                    for j in range(HJ // 2):
                        nc.tensor.matmul(
                            ps[:], ghHL[:, 2 * j:2 * j + 2, 0, tsl],
                            w2b[:, 1, 2 * j:2 * j + 2, :],
                            start=False, stop=False,
                            perf_mode=DR, skip_group_check=True)
                    for j in range(NBJ // 2):
                        nc.tensor.matmul(
                            ps[:], gt[:, 2 * j:2 * j + 2, tsl],
                            vg[:, 2 * j:2 * j + 2, csl],
                            start=False, stop=(j == NBJ // 2 - 1),
                            perf_mode=DR, skip_group_check=True)
                    o_sb = po.tile([P, 256], F32, tag="o_sb")
                    nc.scalar.mul(o_sb[:], ps[:], OUT_SC)
                    nc.sync.dma_start(out_v[:, ti, csl], o_sb[:])
                if c < 2:   # stream chunks 2,3 once 0,1 are consumed
                    w2b = w2p.tile([P, 2, HJ, 256], FP8, tag="w2b")
                    nc.scalar.dma_start(
                        w2b[:].rearrange("p a b c -> p (a b c)"),
                        w2_v[:, c + 2, :])
                    w2tiles.append(w2b)

    nc.compile()
    return nc


_cached_nc = None


def _fp8_split(a, scale):
    hi = (a * scale).astype(FP8NP)
    lo = (a * scale - hi.astype(np.float32)).astype(FP8NP)
    return hi, lo


def _prep_weights(W1, W2, ln_g, ln_b, router_W, router_b, raw_U, raw_V, gamma):
    W1 = np.asarray(W1, np.float32)
    W2 = np.asarray(W2, np.float32)
    ln_g = np.asarray(ln_g, np.float32)
    ln_b = np.asarray(ln_b, np.float32)
    router_W = np.asarray(router_W, np.float32)
    router_b = np.asarray(router_b, np.float32)
    raw_U = np.asarray(raw_U, np.float32)
    raw_V = np.asarray(raw_V, np.float32)
    gam = np.asarray(gamma, np.float32).reshape(D)

    # w1: [(p hj), (two k c)]
    w1hi, w1lo = _fp8_split(W1.T, SW1)                        # [D, H]
    w1s = np.stack([w1hi, w1lo], 0).reshape(2, DK, P, HJ, P)  # 2 k p hj c
    w1s = np.ascontiguousarray(np.transpose(w1s, (2, 3, 0, 1, 4)))
    w1s = w1s.reshape(P * HJ, 2 * DK * P)

    # w2: chunk-major [(p c4), (two hj 256)]
    w2hi, w2lo = _fp8_split(W2.T, SW2)                        # [H, D]
    w2s = np.stack([w2hi, w2lo], 0).reshape(2, HJ, P, 4, 256)  # 2 hj p c d
    w2s = np.ascontiguousarray(np.transpose(w2s, (2, 3, 0, 1, 4)))
    w2s = w2s.reshape(P * 4, 2 * HJ * 256)

    wgm = (router_W * ln_g[None, :]).T                        # [D, NB]
    sg = wgm.sum(axis=0)
    wgp = np.ascontiguousarray(((wgm - sg[None, :] / D) * SWG).astype(FP8NP))
    cvec = ln_b @ router_W.T + router_b
    c16 = np.ascontiguousarray(cvec.astype(BF16NP).reshape(1, NB))

    un = raw_U / np.maximum(np.linalg.norm(raw_U, axis=1, keepdims=True), EPS)
    unp = np.ascontiguousarray((un.T * SU).astype(FP8NP))      # [D, NB]
    vn = raw_V / np.maximum(np.linalg.norm(raw_V, axis=1, keepdims=True), EPS)
    vgp = np.ascontiguousarray((vn * gam[None, :] * SVG).astype(FP8NP))

    eye = np.ascontiguousarray(np.eye(P, dtype=np.float32).astype(BF16NP))
    return {
        "w1": w1s, "w2": w2s, "wg": wgp, "un": unp, "vg": vgp,
        "c16": c16, "eye": eye,
    }


def kernel(x, W1, W2, ln_g, ln_b, router_W, router_b, raw_U, raw_V, gamma):
    global _cached_nc
    x = np.asarray(x, np.float32).reshape(-1, D)

    if _cached_nc is None:
        _cached_nc = _build()
    nc = _cached_nc
    wmap = _prep_weights(W1, W2, ln_g, ln_b, router_W, router_b,
                         raw_U, raw_V, gamma)

    in_maps = []
    for cidx in range(NCORE):
        shard = x[cidx * T:(cidx + 1) * T]                 # [T, D]
        xt = np.ascontiguousarray(shard.T)                 # [D, T]
        x16c = xt.astype(BF16NP)
        xhi = (xt * SX).astype(FP8NP)
        xlo = (xt * SX - xhi.astype(np.float32)).astype(FP8NP)
        xhl = np.empty((2 * DK, P, T), FP8NP)
        xhl[0::2] = xhi.reshape(DK, P, T)
        xhl[1::2] = xlo.reshape(DK, P, T)
        in_maps.append({
            "x16": x16c, "xhl": np.ascontiguousarray(xhl.reshape(2 * D, T)),
            **wmap,
        })
    res = run_bass_kernel_spmd(nc, in_maps, list(range(NCORE)))
    kernel._last_results = res
    out = np.concatenate([res.results[c]["out"] for c in range(NCORE)], axis=0)
    return out.reshape(4, 2048, D)


# revision 16
# speedup vs baseline: 1.5201x; 1.0778x over previous
"""DSC layer (moe_routing) on 8 TRN2 NeuronCores, data-parallel over tokens.

fp8 DoubleRow formulation. All big matmuls run as fp8e4 (e4m3) DoubleRow
pairs (two 128-row k-tiles per PE instruction at 0.5 cycles per output
row) with a hi+lo error-compensation split on the precision-critical
FFN path:

  x   ~= (x_hi + x_lo)/4            (two e4m3 planes, scale 4)
  W1  ~= (W1_hi + W1_lo)/32
  h    = (x_hi+x_lo)@W1_hi [dup-pair]  +  x_hi@W1_lo [tile-pair]
  gh   = gelu(h)   (ACT, f32) -> gh_hi = fp8(gh), gh_lo = fp8(gh - gh_hi)
  W2  ~= (W2_hi + W2_lo)/256
  out  = (gh_hi+gh_lo)@W2_hi + gh_hi@W2_lo + dyn      (PSUM accum, /256)

The dyn path (router top-8 + U/V basis) contributes ~0.17% of the output
norm, so it runs in pure fp8: router logits r0 = x_hi@wg' with the LN
mean-correction folded into wg' (wg' = g*rW - colsum/D), h_full =
x_hi@U_norm, dyn = G@(V_norm*gamma). G is transposed on PE in bf16 and
cast to fp8 at the PSUM evict. LN stats (mu, var) come from [t,1]
matmuls against a ones column (stationary = x bf16, squared in place for
the second moment). tanh(S) is computed as 1 - 2/(exp(2S)+1) so that the
A-phase only ever uses the {exp, ln} activation-table set; all ACT
table switches are batched (2 per token-tile pair instead of ~4).

U/V normalization, router weight folding, transposed layouts, and the
fp8 hi/lo weight splits are host-side prep; only math runs on device.
"""
import sys, os
sys.path.insert(0, "/opt/trn_rl_repo")
from contextlib import ExitStack
import numpy as np
import ml_dtypes
import concourse.bass as bass
import concourse.mybir as mybir
from concourse import bacc
from concourse.tile import TileContext
from concourse.bass_utils import run_bass_kernel_spmd

F32 = mybir.dt.float32
BF16 = mybir.dt.bfloat16
FP8 = mybir.dt.float8e4
AF = mybir.ActivationFunctionType
OP = mybir.AluOpType
DR = mybir.MatmulPerfMode.DoubleRow
FP8NP = ml_dtypes.float8_e4m3
BF16NP = ml_dtypes.bfloat16

D, NB, H = 1024, 512, 4096
NCORE = 8
T = 1024          # tokens per core
P = 128
TI = T // P       # 8 token tiles
DK = D // P       # 8 k-tiles over D
HJ = H // P       # 32 tiles over ffn hidden
NBJ = NB // P     # 4 tiles over basis dim
TAU = 10.0
EPS = 1e-6

SX = 4.0          # x fp8 scale
SW1 = 32.0        # W1 fp8 scale
SWG = 256.0       # router weight fp8 scale
SU = 32.0         # U_norm fp8 scale
SVG = 64.0        # (V_norm*gamma) fp8 scale
SW2 = 256.0       # W2 fp8 scale
GELU_SC = 1.0 / (SX * SW1)                  # FFN1 psum -> true h
RS_BIAS = float(np.log(1.0 / (SX * SWG)))   # fold router psum scale into rs
QF = 4.0 / (SX * SU)                        # fold hf psum scale + G fp8 scale
OUT_SC = 1.0 / SW2                          # FFN2 psum -> true out


def _build():
    nc = bacc.Bacc("TRN2", target_bir_lowering=False, debug=False, num_devices=NCORE)
    x16_e = nc.declare_dram_parameter("x16", [D, T], BF16, isOutput=False)
    xhl_e = nc.declare_dram_parameter("xhl", [2 * D, T], FP8, isOutput=False)
    w1_e = nc.declare_dram_parameter("w1", [P * HJ, 2 * DK * P], FP8, isOutput=False)
    w2_e = nc.declare_dram_parameter("w2", [P * 4, 2 * HJ * 256], FP8, isOutput=False)
    wg_e = nc.declare_dram_parameter("wg", [D, NB], FP8, isOutput=False)
    un_e = nc.declare_dram_parameter("un", [D, NB], FP8, isOutput=False)
    vg_e = nc.declare_dram_parameter("vg", [NB, D], FP8, isOutput=False)
    c16_e = nc.declare_dram_parameter("c16", [1, NB], BF16, isOutput=False)
    eye_e = nc.declare_dram_parameter("eye", [P, P], BF16, isOutput=False)
    out_e = nc.declare_dram_parameter("out", [T, D], F32, isOutput=True)

    x16_v = x16_e[:].rearrange("(k p) t -> p k t", p=P)       # [128, 8, T]
    xhl_v = xhl_e[:].rearrange("(k p) t -> p k t", p=P)       # [128, 16, T]
    w1_v = w1_e[:].rearrange("(p h) x -> p h x", p=P)         # [128, 32, 2048]
    w2_v = w2_e[:].rearrange("(p c) x -> p c x", p=P)         # [128, 4, 16384]
    wg_v = wg_e[:].rearrange("(k p) n -> p k n", p=P)
    un_v = un_e[:].rearrange("(k p) n -> p k n", p=P)
    vg_v = vg_e[:].rearrange("(j p) d -> p j d", p=P)
    out_v = out_e[:].rearrange("(to p) d -> p to d", p=P)

    with TileContext(nc) as tc, ExitStack() as ctx:
        const = ctx.enter_context(tc.tile_pool(name="const", bufs=1))
        persist = ctx.enter_context(tc.tile_pool(name="persist", bufs=1))
        w2p = ctx.enter_context(tc.tile_pool(name="w2p", bufs=2))

        ones_col = const.tile([P, 1], BF16)
        nc.vector.memset(ones_col[:], 1.0)
        ones_row = const.tile([1, P], BF16)
        nc.vector.memset(ones_row[:], 1.0)
        epsb = const.tile([P, 1], F32)
        nc.vector.memset(epsb[:], 1e-5)
        rsbias = const.tile([P, 1], F32)
        nc.vector.memset(rsbias[:], RS_BIAS)
        ident = const.tile([P, P], BF16)
        c_b = const.tile([P, NB], F32)

        xhl = persist.tile([P, 2 * DK, T], FP8)    # 16K/part
        vg = persist.tile([P, NBJ, D], FP8)        # 4K
        gt = persist.tile([P, NBJ, T], FP8)        # 4K (fp8(4*G^T))
        hfall = persist.tile([P, TI, NB], BF16)    # 8K (128*h_lat)
        zsall = persist.tile([P, TI, NB], BF16)    # 8K (masked alpha)
        gall = persist.tile([P, TI, NB], BF16)     # 8K (4*G pre-transpose)
        ghHL = persist.tile([P, HJ, 2, T], FP8)    # 64K (gelu hi/lo planes)
        rs_t = persist.tile([P, TI], F32)
        sall = persist.tile([P, TI], F32)
        thall = persist.tile([P, TI], F32)
        qall = persist.tile([P, TI], F32)

        # xhl planes as [parity, k]: xv2[:, 0, j] = x_hi tile j
        xv2 = xhl[:].rearrange("p (k two) t -> p two k t", two=2)

        ctx2 = ExitStack()
        tabs = ctx2.enter_context(tc.tile_pool(name="tabs", bufs=1))
        w1p = ctx2.enter_context(tc.tile_pool(name="w1p", bufs=5))
        pgh = ctx2.enter_context(tc.tile_pool(name="pgh", bufs=3))
        psF = ctx2.enter_context(tc.tile_pool(name="psF", bufs=3, space="PSUM"))

        # ---------- DMA queue (SP) order: xhl -> w1[0..3] -> wg/un -> x16
        nc.sync.dma_start(xhl[:], xhl_v[:])
        w1tiles = []
        for hj in range(4):
            w1b = w1p.tile([P, 2, DK, P], FP8, tag="w1b")
            nc.sync.dma_start(
                w1b[:].rearrange("p a b c -> p (a b c)"), w1_v[:, hj, :])
            w1tiles.append(w1b)

        def ffn1_hj(hj):
            if hj + 4 < HJ:
                w1b = w1p.tile([P, 2, DK, P], FP8, tag="w1b")
                nc.sync.dma_start(
                    w1b[:].rearrange("p a b c -> p (a b c)"),
                    w1_v[:, hj + 4, :])
                w1tiles.append(w1b)
            w1b = w1tiles[hj]
            for half in range(2):
                hsl = slice(half * 512, (half + 1) * 512)
                ps = psF.tile([P, 512], F32, tag="pF1")
                for j in range(DK):
                    nc.tensor.matmul(
                        ps[:], w1b[:, 0:1, j, :].to_broadcast([P, 2, P]),
                        xhl[:, 2 * j:2 * j + 2, hsl],
                        start=(j == 0), stop=False,
                        perf_mode=DR, skip_group_check=True)
                for j in range(4):
                    nc.tensor.matmul(
                        ps[:], w1b[:, 1, 2 * j:2 * j + 2, :],
                        xv2[:, 0, 2 * j:2 * j + 2, hsl],
                        start=False, stop=(j == 3),
                        perf_mode=DR, skip_group_check=True)
                gh16 = pgh.tile([P, 512], F32, tag="gh16")
                nc.scalar.activation(gh16[:], ps[:], AF.Gelu, scale=GELU_SC)
                nc.gpsimd.tensor_copy(ghHL[:, hj, 0, hsl], gh16[:])
                nc.vector.scalar_tensor_tensor(
                    ghHL[:, hj, 1, hsl], gh16[:], 1.0, ghHL[:, hj, 0, hsl],
                    OP.mult, OP.subtract)

        # ---------- stats scope (closes before A-phase psum pools open)
        with tc.tile_pool(name="pst", bufs=1) as pst, \
             tc.tile_pool(name="psS", bufs=2, space="PSUM") as psS:
            wg = tabs.tile([P, DK, NB], FP8)
            un = tabs.tile([P, DK, NB], FP8)
            nc.sync.dma_start(wg[:], wg_v[:])
            nc.sync.dma_start(un[:], un_v[:])
            x16 = pst.tile([P, DK, T], BF16, tag="x16")
            nc.sync.dma_start(x16[:], x16_v[:])
            c16 = tabs.tile([1, NB], BF16)
            nc.sync.dma_start(c16[:], c16_e[:])
            eyef = tabs.tile([P, P], BF16, tag="eyef")
            nc.sync.dma_start(eyef[:], eye_e[:])
            nc.gpsimd.tensor_copy(ident[:], eyef[:])
            nc.sync.dma_start(vg[:], vg_v[:])

            ffn1_hj(0)
            ffn1_hj(1)
            ffn1_hj(2)

            musq = pst.tile([P, TI, 2], F32, tag="musq")
            for ti in range(TI):
                tsl = slice(ti * P, (ti + 1) * P)
                ps = psS.tile([P, 1], F32, tag="pmu")
                for dk in range(DK):
                    nc.tensor.matmul(ps[:], x16[:, dk, tsl], ones_col[:],
                                     start=(dk == 0), stop=(dk == DK - 1))
                nc.vector.tensor_copy(musq[:, ti, 0:1], ps[:])
            for dk in range(DK):    # square in place
                nc.vector.tensor_tensor(x16[:, dk, :], x16[:, dk, :],
                                        x16[:, dk, :], OP.mult)
            for ti in range(TI):
                tsl = slice(ti * P, (ti + 1) * P)
                ps = psS.tile([P, 1], F32, tag="pmu")
                for dk in range(DK):
                    nc.tensor.matmul(ps[:], x16[:, dk, tsl], ones_col[:],
                                     start=(dk == 0), stop=(dk == DK - 1))
                nc.vector.tensor_copy(musq[:, ti, 1:2], ps[:])
            # c_b broadcast
            cps = psS.tile([P, NB], F32, tag="pcb", bufs=1)
            nc.tensor.matmul(cps[:], ones_row[:], c16[:], start=True, stop=True)
            nc.vector.tensor_copy(c_b[:], cps[:])

            mu_all = pst.tile([P, TI], F32, tag="mu_all")
            sq_all = pst.tile([P, TI], F32, tag="sq_all")
            var_all = pst.tile([P, TI], F32, tag="var_all")
            nc.vector.tensor_scalar_mul(mu_all[:], musq[:, :, 0], 1.0 / D)
            nc.vector.tensor_scalar_mul(sq_all[:], musq[:, :, 1], 1.0 / D)
            nc.vector.tensor_tensor(var_all[:], mu_all[:], mu_all[:], OP.mult)
            nc.vector.tensor_sub(var_all[:], sq_all[:], var_all[:])
            lnv = pst.tile([P, TI], F32, tag="lnv")
            nc.scalar.activation(lnv[:], var_all[:], AF.Ln, bias=epsb[:])
            nc.scalar.activation(rs_t[:], lnv[:], AF.Exp, scale=-0.5,
                                 bias=rsbias[:])

        psA = ctx2.enter_context(tc.tile_pool(name="psA", bufs=2, space="PSUM"))
        psT = ctx2.enter_context(tc.tile_pool(name="psT", bufs=1, space="PSUM"))
        pa = ctx2.enter_context(tc.tile_pool(name="pa", bufs=2))
        pasm = ctx2.enter_context(tc.tile_pool(name="pasm", bufs=3))

        rf_l = [None] * TI

        def emit_A1(ti):
            """Router + h_full matmuls, logit fixup, clip (no ACT tables)."""
            tsl = slice(ti * P, (ti + 1) * P)
            r0 = psA.tile([P, NB], F32, tag="pArt")
            for nbc in range(2):
                nsl = slice(nbc * 256, (nbc + 1) * 256)
                for j in range(4):
                    nc.tensor.matmul(
                        r0[:, nsl], xv2[:, 0, 2 * j:2 * j + 2, tsl],
                        wg[:, 2 * j:2 * j + 2, nsl],
                        start=(nbc == 0 and j == 0),
                        stop=(nbc == 1 and j == 3),
                        perf_mode=DR, skip_group_check=True)
            rf = pa.tile([P, NB], F32, tag="rf", bufs=TI)
            nc.vector.scalar_tensor_tensor(
                rf[:], r0[:], rs_t[:, ti:ti + 1], c_b[:], OP.mult, OP.add)
            nc.gpsimd.tensor_scalar(rf[:], rf[:], TAU, -TAU, OP.min, OP.max)
            rf_l[ti] = rf
            hf = psA.tile([P, NB], F32, tag="pAhf")
            for nbc in range(2):
                nsl = slice(nbc * 256, (nbc + 1) * 256)
                for j in range(4):
                    nc.tensor.matmul(
                        hf[:, nsl], xv2[:, 0, 2 * j:2 * j + 2, tsl],
                        un[:, 2 * j:2 * j + 2, nsl],
                        start=(nbc == 0 and j == 0),
                        stop=(nbc == 1 and j == 3),
                        perf_mode=DR, skip_group_check=True)
            nc.scalar.copy(hfall[:, ti, :], hf[:])

        def emit_A2():
            """Batched softplus/top-8/q/G for all tiles: ACT runs one Exp
            block, one Ln block, one Exp(2S) block (3 table loads total).
            tanh(S) = 1 - 2/(exp(2S)+1). Activations run in place on rf."""
            for ti in range(TI):
                nc.scalar.activation(rf_l[ti][:], rf_l[ti][:], AF.Exp)
            for ti in range(TI):
                nc.scalar.activation(rf_l[ti][:], rf_l[ti][:], AF.Ln, bias=1.0)
            for ti in range(TI):
                alpha = rf_l[ti]
                m8 = pasm.tile([P, 8], F32, tag="m8")
                nc.vector.max(out=m8[:], in_=alpha[:])
                nc.vector.reduce_sum(sall[:, ti:ti + 1], m8[:],
                                     axis=mybir.AxisListType.X)
                repl = pa.tile([P, NB], F32, tag="repl")
                nc.vector.match_replace(out=repl[:], in_to_replace=m8[:],
                                        in_values=alpha[:], imm_value=0.0)
                nc.gpsimd.tensor_tensor(zsall[:, ti, :], alpha[:], repl[:],
                                        OP.subtract)
            e2s = pasm.tile([P, TI], F32, tag="e2s")
            nc.scalar.activation(e2s[:], sall[:], AF.Exp, scale=2.0)
            nc.vector.tensor_scalar_add(e2s[:], e2s[:], 1.0)
            nc.vector.reciprocal(e2s[:], e2s[:])
            nc.vector.tensor_scalar(thall[:], e2s[:], -2.0, 1.0,
                                    OP.mult, OP.add)
            sp = pasm.tile([P, TI], F32, tag="sp")
            nc.vector.tensor_scalar_add(sp[:], sall[:], EPS)
            nc.vector.reciprocal(sp[:], sp[:])
            nc.vector.scalar_tensor_tensor(
                qall[:], thall[:], QF, sp[:], OP.mult, OP.mult)
            for ti in range(TI):
                nc.vector.scalar_tensor_tensor(
                    gall[:, ti, :], zsall[:, ti, :], qall[:, ti:ti + 1],
                    hfall[:, ti, :], OP.mult, OP.mult)

        def emit_T(ti):
            tsl = slice(ti * P, (ti + 1) * P)
            for nbj in range(NBJ):
                pt = psT.tile([P, P], BF16, tag="pt")
                nc.tensor.transpose(
                    pt[:], gall[:, ti, nbj * P:(nbj + 1) * P], ident[:])
                nc.vector.tensor_copy(gt[:, nbj, tsl], pt[:])

        # A1 at hj 3..10; batched A2 at hj 11; transposes at hj 14..21.
        w2tiles = []
        for hj in range(3, HJ):
            if 3 <= hj < 11:
                emit_A1(hj - 3)
            if hj == 11:
                emit_A2()
            if 14 <= hj < 22:
                emit_T(hj - 14)
            if hj == 3 or hj == 5:
                w2b = w2p.tile([P, 2, HJ, 256], FP8, tag="w2b")
                nc.sync.dma_start(
                    w2b[:].rearrange("p a b c -> p (a b c)"),
                    w2_v[:, (hj - 3) // 2, :])
                w2tiles.append(w2b)
            ffn1_hj(hj)
        ctx2.close()

        # ---------------- FFN2 + dyn ----------------
        with tc.tile_pool(name="po", bufs=3) as po, \
             tc.tile_pool(name="psO", bufs=4, space="PSUM") as psO:
            for c in range(4):
                csl = slice(c * 256, (c + 1) * 256)
                w2b = w2tiles[c]
                for ti in range(TI):
                    tsl = slice(ti * P, (ti + 1) * P)
                    ps = psO.tile([P, 256], F32, tag="pO")
                    for hj in range(HJ):
                        nc.tensor.matmul(
                            ps[:], ghHL[:, hj, :, tsl],
                            w2b[:, 0:1, hj, :].to_broadcast([P, 2, 256]),
                            start=(hj == 0), stop=False,
                            perf_mode=DR, skip_group_check=True)
                    for j in range(HJ // 2):
                        nc.tensor.matmul(
                            ps[:], ghHL[:, 2 * j:2 * j + 2, 0, tsl],
                            w2b[:, 1, 2 * j:2 * j + 2, :],
                            start=False, stop=False,
                            perf_mode=DR, skip_group_check=True)
                    for j in range(NBJ // 2):
                        nc.tensor.matmul(
                            ps[:], gt[:, 2 * j:2 * j + 2, tsl],
                            vg[:, 2 * j:2 * j + 2, csl],
                            start=False, stop=(j == NBJ // 2 - 1),
                            perf_mode=DR, skip_group_check=True)
                    o_sb = po.tile([P, 256], F32, tag="o_sb")
                    nc.scalar.mul(o_sb[:], ps[:], OUT_SC)
                    nc.sync.dma_start(out_v[:, ti, csl], o_sb[:])
                if c < 2:   # stream chunks 2,3 once 0,1 are consumed
                    w2b = w2p.tile([P, 2, HJ, 256], FP8, tag="w2b")
                    nc.sync.dma_start(
                        w2b[:].rearrange("p a b c -> p (a b c)"),
                        w2_v[:, c + 2, :])
                    w2tiles.append(w2b)

    nc.compile()
    return nc


_cached_nc = None


def _fp8_split(a, scale):
    hi = (a * scale).astype(FP8NP)
    lo = (a * scale - hi.astype(np.float32)).astype(FP8NP)
    return hi, lo


def _prep_weights(W1, W2, ln_g, ln_b, router_W, router_b, raw_U, raw_V, gamma):
    W1 = np.asarray(W1, np.float32)
    W2 = np.asarray(W2, np.float32)
    ln_g = np.asarray(ln_g, np.float32)
    ln_b = np.asarray(ln_b, np.float32)
    router_W = np.asarray(router_W, np.float32)
    router_b = np.asarray(router_b, np.float32)
    raw_U = np.asarray(raw_U, np.float32)
    raw_V = np.asarray(raw_V, np.float32)
    gam = np.asarray(gamma, np.float32).reshape(D)

    # w1: [(p hj), (two k c)]
    w1hi, w1lo = _fp8_split(W1.T, SW1)                        # [D, H]
    w1s = np.stack([w1hi, w1lo], 0).reshape(2, DK, P, HJ, P)  # 2 k p hj c
    w1s = np.ascontiguousarray(np.transpose(w1s, (2, 3, 0, 1, 4)))
    w1s = w1s.reshape(P * HJ, 2 * DK * P)

    # w2: chunk-major [(p c4), (two hj 256)]
    w2hi, w2lo = _fp8_split(W2.T, SW2)                        # [H, D]
    w2s = np.stack([w2hi, w2lo], 0).reshape(2, HJ, P, 4, 256)  # 2 hj p c d
    w2s = np.ascontiguousarray(np.transpose(w2s, (2, 3, 0, 1, 4)))
    w2s = w2s.reshape(P * 4, 2 * HJ * 256)

    wgm = (router_W * ln_g[None, :]).T                        # [D, NB]
    sg = wgm.sum(axis=0)
    wgp = np.ascontiguousarray(((wgm - sg[None, :] / D) * SWG).astype(FP8NP))
    cvec = ln_b @ router_W.T + router_b
    c16 = np.ascontiguousarray(cvec.astype(BF16NP).reshape(1, NB))

    un = raw_U / np.maximum(np.linalg.norm(raw_U, axis=1, keepdims=True), EPS)
    unp = np.ascontiguousarray((un.T * SU).astype(FP8NP))      # [D, NB]
    vn = raw_V / np.maximum(np.linalg.norm(raw_V, axis=1, keepdims=True), EPS)
    vgp = np.ascontiguousarray((vn * gam[None, :] * SVG).astype(FP8NP))

    eye = np.ascontiguousarray(np.eye(P, dtype=np.float32).astype(BF16NP))
    return {
        "w1": w1s, "w2": w2s, "wg": wgp, "un": unp, "vg": vgp,
        "c16": c16, "eye": eye,
    }


def kernel(x, W1, W2, ln_g, ln_b, router_W, router_b, raw_U, raw_V, gamma):
    global _cached_nc
    x = np.asarray(x, np.float32).reshape(-1, D)

    if _cached_nc is None:
        _cached_nc = _build()
    nc = _cached_nc
    wmap = _prep_weights(W1, W2, ln_g, ln_b, router_W, router_b,
                         raw_U, raw_V, gamma)

    in_maps = []
    for cidx in range(NCORE):
        shard = x[cidx * T:(cidx + 1) * T]                 # [T, D]
        xt = np.ascontiguousarray(shard.T)                 # [D, T]
        x16c = xt.astype(BF16NP)
        xhi = (xt * SX).astype(FP8NP)
        xlo = (xt * SX - xhi.astype(np.float32)).astype(FP8NP)
        xhl = np.empty((2 * DK, P, T), FP8NP)
        xhl[0::2] = xhi.reshape(DK, P, T)
        xhl[1::2] = xlo.reshape(DK, P, T)
        in_maps.append({
            "x16": x16c, "xhl": np.ascontiguousarray(xhl.reshape(2 * D, T)),
            **wmap,
        })
    res = run_bass_kernel_spmd(nc, in_maps, list(range(NCORE)))
    kernel._last_results = res
    out = np.concatenate([res.results[c]["out"] for c in range(NCORE)], axis=0)
    return out.reshape(4, 2048, D)


# revision 22
# speedup vs baseline: 1.5608x; 1.0267x over previous
"""DSC layer (moe_routing) on 8 TRN2 NeuronCores, data-parallel over tokens.

fp8 DoubleRow formulation. All big matmuls run as fp8e4 (e4m3) DoubleRow
pairs (two 128-row k-tiles per PE instruction at 0.5 cycles per output
row) with a hi+lo error-compensation split on the precision-critical
FFN path:

  x   ~= (x_hi + x_lo)/4            (two e4m3 planes, scale 4)
  W1  ~= (W1_hi + W1_lo)/32
  h    = (x_hi+x_lo)@W1_hi [dup-pair]  +  x_hi@W1_lo [tile-pair]
  gh   = gelu(h)   (ACT, f32) -> gh_hi = fp8(gh), gh_lo = fp8(gh - gh_hi)
  W2  ~= (W2_hi + W2_lo)/256
  out  = (gh_hi+gh_lo)@W2_hi + gh_hi@W2_lo + dyn      (PSUM accum, /256)

The dyn path (router top-8 + U/V basis) contributes ~0.17% of the output
norm, so it runs in pure fp8: router logits r0 = x_hi@wg' with the LN
mean-correction folded into wg' (wg' = g*rW - colsum/D), h_full =
x_hi@U_norm, dyn = G@(V_norm*gamma). G is transposed on PE in bf16 and
cast to fp8 at the PSUM evict. LN stats (mu, var) come from [t,1]
matmuls against a ones column (stationary = x bf16, squared in place for
the second moment). tanh(S) is computed as 1 - 2/(exp(2S)+1) so that the
A-phase only ever uses the {exp, ln} activation-table set; all ACT
table switches are batched (2 per token-tile pair instead of ~4).

U/V normalization, router weight folding, transposed layouts, and the
fp8 hi/lo weight splits are host-side prep; only math runs on device.
"""
import sys, os
sys.path.insert(0, "/opt/trn_rl_repo")
from contextlib import ExitStack
import numpy as np
import ml_dtypes
import concourse.bass as bass
import concourse.mybir as mybir
from concourse import bacc
from concourse.tile import TileContext
from concourse.bass_utils import run_bass_kernel_spmd

F32 = mybir.dt.float32
BF16 = mybir.dt.bfloat16
FP8 = mybir.dt.float8e4
AF = mybir.ActivationFunctionType
OP = mybir.AluOpType
DR = mybir.MatmulPerfMode.DoubleRow
FP8NP = ml_dtypes.float8_e4m3
BF16NP = ml_dtypes.bfloat16

D, NB, H = 1024, 512, 4096
NCORE = 8
T = 1024          # tokens per core
P = 128
TI = T // P       # 8 token tiles
DK = D // P       # 8 k-tiles over D
HJ = H // P       # 32 tiles over ffn hidden
NBJ = NB // P     # 4 tiles over basis dim
TAU = 10.0
EPS = 1e-6

SX = 4.0          # x fp8 scale
SW1 = 32.0        # W1 fp8 scale
SWG = 256.0       # router weight fp8 scale
SU = 32.0         # U_norm fp8 scale
SVG = 64.0        # (V_norm*gamma) fp8 scale
SW2 = 256.0       # W2 fp8 scale
GELU_SC = 1.0 / (SX * SW1)                  # FFN1 psum -> true h
RS_BIAS = float(np.log(1.0 / (SX * SWG)))   # fold router psum scale into rs
QF = 4.0 / (SX * SU)                        # fold hf psum scale + G fp8 scale
OUT_SC = 1.0 / SW2                          # FFN2 psum -> true out


def _build():
    nc = bacc.Bacc("TRN2", target_bir_lowering=False, debug=False, num_devices=NCORE)
    x16_e = nc.declare_dram_parameter("x16", [D, T], BF16, isOutput=False)
    xhl_e = nc.declare_dram_parameter("xhl", [2 * D, T], FP8, isOutput=False)
    w1_e = nc.declare_dram_parameter("w1", [P * HJ, 2 * DK * P], FP8, isOutput=False)
    w2_e = nc.declare_dram_parameter("w2", [P * 4, 2 * HJ * 256], FP8, isOutput=False)
    wg_e = nc.declare_dram_parameter("wg", [D, NB], FP8, isOutput=False)
    un_e = nc.declare_dram_parameter("un", [D, NB], FP8, isOutput=False)
    vg_e = nc.declare_dram_parameter("vg", [NB, D], FP8, isOutput=False)
    c16_e = nc.declare_dram_parameter("c16", [1, NB], BF16, isOutput=False)
    eye_e = nc.declare_dram_parameter("eye", [P, P], BF16, isOutput=False)
    out_e = nc.declare_dram_parameter("out", [T, D], F32, isOutput=True)

    x16_v = x16_e[:].rearrange("(k p) t -> p k t", p=P)       # [128, 8, T]
    xhl_v = xhl_e[:].rearrange("(k p) t -> p k t", p=P)       # [128, 16, T]
    w1_v = w1_e[:].rearrange("(p h) x -> p h x", p=P)         # [128, 32, 2048]
    w2_v = w2_e[:].rearrange("(p c) x -> p c x", p=P)         # [128, 4, 16384]
    wg_v = wg_e[:].rearrange("(k p) n -> p k n", p=P)
    un_v = un_e[:].rearrange("(k p) n -> p k n", p=P)
    vg_v = vg_e[:].rearrange("(j p) d -> p j d", p=P)
    out_v = out_e[:].rearrange("(to p) d -> p to d", p=P)

    with TileContext(nc) as tc, ExitStack() as ctx:
        const = ctx.enter_context(tc.tile_pool(name="const", bufs=1))
        persist = ctx.enter_context(tc.tile_pool(name="persist", bufs=1))
        w2p = ctx.enter_context(tc.tile_pool(name="w2p", bufs=2))

        ones_col = const.tile([P, 1], BF16)
        nc.vector.memset(ones_col[:], 1.0)
        ones_row = const.tile([1, P], BF16)
        nc.vector.memset(ones_row[:], 1.0)
        epsb = const.tile([P, 1], F32)
        nc.vector.memset(epsb[:], 1e-5)
        rsbias = const.tile([P, 1], F32)
        nc.vector.memset(rsbias[:], RS_BIAS)
        ident = const.tile([P, P], BF16)
        c_b = const.tile([P, NB], F32)

        xhl = persist.tile([P, 2 * DK, T], FP8)    # 16K/part
        vg = persist.tile([P, NBJ, D], FP8)        # 4K
        gt = persist.tile([P, NBJ, T], FP8)        # 4K (fp8(4*G^T))
        hfall = persist.tile([P, TI, NB], BF16)    # 8K (128*h_lat)
        zsall = persist.tile([P, TI, NB], BF16)    # 8K (masked alpha)
        gall = persist.tile([P, TI, NB], BF16)     # 8K (4*G pre-transpose)
        ghHL = persist.tile([P, HJ, 2, T], FP8)    # 64K (gelu hi/lo planes)
        rs_t = persist.tile([P, TI], F32)
        sall = persist.tile([P, TI], F32)
        thall = persist.tile([P, TI], F32)
        qall = persist.tile([P, TI], F32)

        # xhl planes as [parity, k]: xv2[:, 0, j] = x_hi tile j
        xv2 = xhl[:].rearrange("p (k two) t -> p two k t", two=2)

        ctx2 = ExitStack()
        tabs = ctx2.enter_context(tc.tile_pool(name="tabs", bufs=1))
        w1p = ctx2.enter_context(tc.tile_pool(name="w1p", bufs=5))
        pgh = ctx2.enter_context(tc.tile_pool(name="pgh", bufs=3))
        psF = ctx2.enter_context(tc.tile_pool(name="psF", bufs=3, space="PSUM"))

        # ---------- DMA queue (SP) order: xhl -> w1[0..3] -> wg/un -> x16
        nc.sync.dma_start(xhl[:], xhl_v[:])
        w1tiles = []
        for hj in range(4):
            w1b = w1p.tile([P, 2, DK, P], FP8, tag="w1b")
            nc.sync.dma_start(
                w1b[:].rearrange("p a b c -> p (a b c)"), w1_v[:, hj, :])
            w1tiles.append(w1b)

        psF2 = [None]

        def ffn1_hj(hj):
            if hj + 4 < HJ:
                w1b = w1p.tile([P, 2, DK, P], FP8, tag="w1b")
                nc.sync.dma_start(
                    w1b[:].rearrange("p a b c -> p (a b c)"),
                    w1_v[:, hj + 4, :])
                w1tiles.append(w1b)
            w1b = w1tiles[hj]
            for half in range(2):
                hsl = slice(half * 512, (half + 1) * 512)
                pool = psF2[0] if (psF2[0] is not None and half == 1) else psF
                ps = pool.tile([P, 512], F32, tag="pF1")
                for j in range(DK):
                    nc.tensor.matmul(
                        ps[:], w1b[:, 0:1, j, :].to_broadcast([P, 2, P]),
                        xhl[:, 2 * j:2 * j + 2, hsl],
                        start=(j == 0), stop=False,
                        perf_mode=DR, skip_group_check=True)
                for j in range(4):
                    nc.tensor.matmul(
                        ps[:], w1b[:, 1, 2 * j:2 * j + 2, :],
                        xv2[:, 0, 2 * j:2 * j + 2, hsl],
                        start=False, stop=(j == 3),
                        perf_mode=DR, skip_group_check=True)
                gh16 = pgh.tile([P, 512], F32, tag="gh16")
                nc.scalar.activation(gh16[:], ps[:], AF.Gelu, scale=GELU_SC)
                nc.gpsimd.tensor_copy(ghHL[:, hj, 0, hsl], gh16[:])
                nc.vector.scalar_tensor_tensor(
                    ghHL[:, hj, 1, hsl], gh16[:], 1.0, ghHL[:, hj, 0, hsl],
                    OP.mult, OP.subtract)

        # ---------- stats scope (closes before A-phase psum pools open)
        with tc.tile_pool(name="pst", bufs=1) as pst, \
             tc.tile_pool(name="psS", bufs=2, space="PSUM") as psS:
            wg = tabs.tile([P, DK, NB], FP8)
            un = tabs.tile([P, DK, NB], FP8)
            nc.sync.dma_start(wg[:], wg_v[:])
            nc.sync.dma_start(un[:], un_v[:])
            x16 = pst.tile([P, DK, T], BF16, tag="x16")
            nc.sync.dma_start(x16[:], x16_v[:])
            c16 = tabs.tile([1, NB], BF16)
            nc.sync.dma_start(c16[:], c16_e[:])
            eyef = tabs.tile([P, P], BF16, tag="eyef")
            nc.sync.dma_start(eyef[:], eye_e[:])
            nc.gpsimd.tensor_copy(ident[:], eyef[:])
            nc.sync.dma_start(vg[:], vg_v[:])

            ffn1_hj(0)
            ffn1_hj(1)
            ffn1_hj(2)

            musq = pst.tile([P, TI, 2], F32, tag="musq")
            for ti in range(TI):
                tsl = slice(ti * P, (ti + 1) * P)
                ps = psS.tile([P, 1], F32, tag="pmu")
                for dk in range(DK):
                    nc.tensor.matmul(ps[:], x16[:, dk, tsl], ones_col[:],
                                     start=(dk == 0), stop=(dk == DK - 1))
                nc.vector.tensor_copy(musq[:, ti, 0:1], ps[:])
            for dk in range(DK):    # square in place
                nc.vector.tensor_tensor(x16[:, dk, :], x16[:, dk, :],
                                        x16[:, dk, :], OP.mult)
            for ti in range(TI):
                tsl = slice(ti * P, (ti + 1) * P)
                ps = psS.tile([P, 1], F32, tag="pmu")
                for dk in range(DK):
                    nc.tensor.matmul(ps[:], x16[:, dk, tsl], ones_col[:],
                                     start=(dk == 0), stop=(dk == DK - 1))
                nc.vector.tensor_copy(musq[:, ti, 1:2], ps[:])
            # c_b broadcast
            cps = psS.tile([P, NB], F32, tag="pcb", bufs=1)
            nc.tensor.matmul(cps[:], ones_row[:], c16[:], start=True, stop=True)
            nc.vector.tensor_copy(c_b[:], cps[:])

            mu_all = pst.tile([P, TI], F32, tag="mu_all")
            sq_all = pst.tile([P, TI], F32, tag="sq_all")
            var_all = pst.tile([P, TI], F32, tag="var_all")
            nc.vector.tensor_scalar_mul(mu_all[:], musq[:, :, 0], 1.0 / D)
            nc.vector.tensor_scalar_mul(sq_all[:], musq[:, :, 1], 1.0 / D)
            nc.vector.tensor_tensor(var_all[:], mu_all[:], mu_all[:], OP.mult)
            nc.vector.tensor_sub(var_all[:], sq_all[:], var_all[:])
            lnv = pst.tile([P, TI], F32, tag="lnv")
            nc.scalar.activation(lnv[:], var_all[:], AF.Ln, bias=epsb[:])
            nc.scalar.activation(rs_t[:], lnv[:], AF.Exp, scale=-0.5,
                                 bias=rsbias[:])

        psT = ctx2.enter_context(tc.tile_pool(name="psT", bufs=1, space="PSUM"))
        pa = ctx2.enter_context(tc.tile_pool(name="pa", bufs=2))
        pasm = ctx2.enter_context(tc.tile_pool(name="pasm", bufs=3))
        ctxA = ExitStack()
        psA = ctxA.enter_context(tc.tile_pool(name="psA", bufs=2, space="PSUM"))

        rf_l = [None] * TI

        def emit_A1(ti):
            """Router + h_full matmuls, logit fixup, clip (no ACT tables)."""
            tsl = slice(ti * P, (ti + 1) * P)
            r0 = psA.tile([P, NB], F32, tag="pArt")
            for nbc in range(2):
                nsl = slice(nbc * 256, (nbc + 1) * 256)
                for j in range(4):
                    nc.tensor.matmul(
                        r0[:, nsl], xv2[:, 0, 2 * j:2 * j + 2, tsl],
                        wg[:, 2 * j:2 * j + 2, nsl],
                        start=(nbc == 0 and j == 0),
                        stop=(nbc == 1 and j == 3),
                        perf_mode=DR, skip_group_check=True)
            rf = pa.tile([P, NB], F32, tag="rf", bufs=TI)
            nc.vector.scalar_tensor_tensor(
                rf[:], r0[:], rs_t[:, ti:ti + 1], c_b[:], OP.mult, OP.add)
            nc.gpsimd.tensor_scalar(rf[:], rf[:], TAU, -TAU, OP.min, OP.max)
            rf_l[ti] = rf
            hf = psA.tile([P, NB], F32, tag="pAhf")
            for nbc in range(2):
                nsl = slice(nbc * 256, (nbc + 1) * 256)
                for j in range(4):
                    nc.tensor.matmul(
                        hf[:, nsl], xv2[:, 0, 2 * j:2 * j + 2, tsl],
                        un[:, 2 * j:2 * j + 2, nsl],
                        start=(nbc == 0 and j == 0),
                        stop=(nbc == 1 and j == 3),
                        perf_mode=DR, skip_group_check=True)
            nc.scalar.copy(hfall[:, ti, :], hf[:])

        def emit_A2(tis):
            """Batched softplus/top-8 for a group of tiles: ACT runs one Exp
            block then one Ln block (2 table loads). Activations in place."""
            for ti in tis:
                nc.scalar.activation(rf_l[ti][:], rf_l[ti][:], AF.Exp)
            for ti in tis:
                nc.scalar.activation(rf_l[ti][:], rf_l[ti][:], AF.Ln, bias=1.0)
            for ti in tis:
                alpha = rf_l[ti]
                m8 = pasm.tile([P, 8], F32, tag="m8")
                nc.vector.max(out=m8[:], in_=alpha[:])
                nc.vector.reduce_sum(sall[:, ti:ti + 1], m8[:],
                                     axis=mybir.AxisListType.X)
                repl = pa.tile([P, NB], F32, tag="repl")
                nc.vector.match_replace(out=repl[:], in_to_replace=m8[:],
                                        in_values=alpha[:], imm_value=0.0)
                nc.gpsimd.tensor_tensor(zsall[:, ti, :], alpha[:], repl[:],
                                        OP.subtract)

        def emit_A3():
            """tanh(S) = 1 - 2/(exp(2S)+1), q, and G for all tiles."""
            e2s = pasm.tile([P, TI], F32, tag="e2s")
            nc.scalar.activation(e2s[:], sall[:], AF.Exp, scale=2.0)
            nc.vector.tensor_scalar_add(e2s[:], e2s[:], 1.0)
            nc.vector.reciprocal(e2s[:], e2s[:])
            nc.vector.tensor_scalar(thall[:], e2s[:], -2.0, 1.0,
                                    OP.mult, OP.add)
            sp = pasm.tile([P, TI], F32, tag="sp")
            nc.vector.tensor_scalar_add(sp[:], sall[:], EPS)
            nc.vector.reciprocal(sp[:], sp[:])
            nc.vector.scalar_tensor_tensor(
                qall[:], thall[:], QF, sp[:], OP.mult, OP.mult)
            for ti in range(TI):
                nc.vector.scalar_tensor_tensor(
                    gall[:, ti, :], zsall[:, ti, :], qall[:, ti:ti + 1],
                    hfall[:, ti, :], OP.mult, OP.mult)

        def emit_T(ti):
            tsl = slice(ti * P, (ti + 1) * P)
            for nbj in range(NBJ):
                pt = psT.tile([P, P], BF16, tag="pt")
                nc.tensor.transpose(
                    pt[:], gall[:, ti, nbj * P:(nbj + 1) * P], ident[:])
                nc.vector.tensor_copy(gt[:, nbj, tsl], pt[:])

        # A1 at hj 3..10; A2 quads at hj 11, 13; A3 at 15; transposes 16..23.
        w2tiles = []
        for hj in range(3, 11):
            emit_A1(hj - 3)
            if hj == 3 or hj == 5:
                w2b = w2p.tile([P, 2, HJ, 256], FP8, tag="w2b")
                nc.sync.dma_start(
                    w2b[:].rearrange("p a b c -> p (a b c)"),
                    w2_v[:, (hj - 3) // 2, :])
                w2tiles.append(w2b)
            ffn1_hj(hj)
        ctxA.close()
        psF2[0] = ctx2.enter_context(
            tc.tile_pool(name="psFX", bufs=4, space="PSUM"))
        for hj in range(11, HJ):
            if hj == 11:
                emit_A2(range(0, 4))
            if hj == 13:
                emit_A2(range(4, TI))
            if hj == 15:
                emit_A3()
            if 16 <= hj < 24:
                emit_T(hj - 16)
            ffn1_hj(hj)
        ctx2.close()

        # ---------------- FFN2 + dyn ----------------
        with tc.tile_pool(name="po", bufs=3) as po, \
             tc.tile_pool(name="psO", bufs=4, space="PSUM") as psO:
            for c in range(4):
                csl = slice(c * 256, (c + 1) * 256)
                w2b = w2tiles[c]
                for ti in range(TI):
                    tsl = slice(ti * P, (ti + 1) * P)
                    ps = psO.tile([P, 256], F32, tag="pO")
                    for hj in range(HJ):
                        nc.tensor.matmul(
                            ps[:], ghHL[:, hj, :, tsl],
                            w2b[:, 0:1, hj, :].to_broadcast([P, 2, 256]),
                            start=(hj == 0), stop=False,
                            perf_mode=DR, skip_group_check=True)
                    for j in range(HJ // 2):
                        nc.tensor.matmul(
                            ps[:], ghHL[:, 2 * j:2 * j + 2, 0, tsl],
                            w2b[:, 1, 2 * j:2 * j + 2, :],
                            start=False, stop=False,
                            perf_mode=DR, skip_group_check=True)
                    for j in range(NBJ // 2):
                        nc.tensor.matmul(
                            ps[:], gt[:, 2 * j:2 * j + 2, tsl],
                            vg[:, 2 * j:2 * j + 2, csl],
                            start=False, stop=(j == NBJ // 2 - 1),
                            perf_mode=DR, skip_group_check=True)
                    o_sb = po.tile([P, 256], F32, tag="o_sb")
                    nc.scalar.mul(o_sb[:], ps[:], OUT_SC)
                    nc.sync.dma_start(out_v[:, ti, csl], o_sb[:])
                if c < 2:   # stream chunks 2,3 once 0,1 are consumed
                    w2b = w2p.tile([P, 2, HJ, 256], FP8, tag="w2b")
                    nc.sync.dma_start(
                        w2b[:].rearrange("p a b c -> p (a b c)"),
                        w2_v[:, c + 2, :])
                    w2tiles.append(w2b)

    nc.compile()
    return nc


_cached_nc = None


def _fp8_split(a, scale):
    hi = (a * scale).astype(FP8NP)
    lo = (a * scale - hi.astype(np.float32)).astype(FP8NP)
    return hi, lo


def _prep_weights(W1, W2, ln_g, ln_b, router_W, router_b, raw_U, raw_V, gamma):
    W1 = np.asarray(W1, np.float32)
    W2 = np.asarray(W2, np.float32)
    ln_g = np.asarray(ln_g, np.float32)
    ln_b = np.asarray(ln_b, np.float32)
    router_W = np.asarray(router_W, np.float32)
    router_b = np.asarray(router_b, np.float32)
    raw_U = np.asarray(raw_U, np.float32)
    raw_V = np.asarray(raw_V, np.float32)
    gam = np.asarray(gamma, np.float32).reshape(D)

    # w1: [(p hj), (two k c)]
    w1hi, w1lo = _fp8_split(W1.T, SW1)                        # [D, H]
    w1s = np.stack([w1hi, w1lo], 0).reshape(2, DK, P, HJ, P)  # 2 k p hj c
    w1s = np.ascontiguousarray(np.transpose(w1s, (2, 3, 0, 1, 4)))
    w1s = w1s.reshape(P * HJ, 2 * DK * P)

    # w2: chunk-major [(p c4), (two hj 256)]
    w2hi, w2lo = _fp8_split(W2.T, SW2)                        # [H, D]
    w2s = np.stack([w2hi, w2lo], 0).reshape(2, HJ, P, 4, 256)  # 2 hj p c d
    w2s = np.ascontiguousarray(np.transpose(w2s, (2, 3, 0, 1, 4)))
    w2s = w2s.reshape(P * 4, 2 * HJ * 256)

    wgm = (router_W * ln_g[None, :]).T                        # [D, NB]
    sg = wgm.sum(axis=0)
    wgp = np.ascontiguousarray(((wgm - sg[None, :] / D) * SWG).astype(FP8NP))
    cvec = ln_b @ router_W.T + router_b
    c16 = np.ascontiguousarray(cvec.astype(BF16NP).reshape(1, NB))

    un = raw_U / np.maximum(np.linalg.norm(raw_U, axis=1, keepdims=True), EPS)
    unp = np.ascontiguousarray((un.T * SU).astype(FP8NP))      # [D, NB]
    vn = raw_V / np.maximum(np.linalg.norm(raw_V, axis=1, keepdims=True), EPS)
    vgp = np.ascontiguousarray((vn * gam[None, :] * SVG).astype(FP8NP))

    eye = np.ascontiguousarray(np.eye(P, dtype=np.float32).astype(BF16NP))
    return {
        "w1": w1s, "w2": w2s, "wg": wgp, "un": unp, "vg": vgp,
        "c16": c16, "eye": eye,
    }


def kernel(x, W1, W2, ln_g, ln_b, router_W, router_b, raw_U, raw_V, gamma):
    global _cached_nc
    x = np.asarray(x, np.float32).reshape(-1, D)

    if _cached_nc is None:
        _cached_nc = _build()
    nc = _cached_nc
    wmap = _prep_weights(W1, W2, ln_g, ln_b, router_W, router_b,
                         raw_U, raw_V, gamma)

    in_maps = []
    for cidx in range(NCORE):
        shard = x[cidx * T:(cidx + 1) * T]                 # [T, D]
        xt = np.ascontiguousarray(shard.T)                 # [D, T]
        x16c = xt.astype(BF16NP)
        xhi = (xt * SX).astype(FP8NP)
        xlo = (xt * SX - xhi.astype(np.float32)).astype(FP8NP)
        xhl = np.empty((2 * DK, P, T), FP8NP)
        xhl[0::2] = xhi.reshape(DK, P, T)
        xhl[1::2] = xlo.reshape(DK, P, T)
        in_maps.append({
            "x16": x16c, "xhl": np.ascontiguousarray(xhl.reshape(2 * D, T)),
            **wmap,
        })
    res = run_bass_kernel_spmd(nc, in_maps, list(range(NCORE)))
    kernel._last_results = res
    out = np.concatenate([res.results[c]["out"] for c in range(NCORE)], axis=0)
    return out.reshape(4, 2048, D)
